# revision 1
# baseline (speedup 1.0000x reference)
"""CLIP encoder layer on 8 trn2 NeuronCores, pure data parallel over batch.

Layout strategy (per core, batch shard of 64 sequences = 4928 tokens):
  - x arrives token-major [T, 768] fp32.
  - LayerNorm runs token-major (tokens on partitions, bn_stats/bn_aggr),
    LN scale/bias folded into the downstream projection weights host-side.
  - Normalized activations are PE-transposed (bf16) to feature-major
    [768, N] for the projections (weights stationary, activations moving).
  - Attention per sequence (S=77): scores = qT.T @ kT per head directly in
    feature-major; softmax along free axis (no max subtraction - scores are
    bounded ~|2.5|; causal mask applied multiplicatively after exp);
    attn and v are PE-transposed per head for the ctx matmul.
  - O-projection and FC2 run with swapped operands (activations stationary)
    so their outputs come out token-major, letting the residual adds and the
    second LayerNorm stay token-major with no full-tensor transposes.
  - All matmuls in bf16 (fp32 PSUM accumulation); fp32 elsewhere.
    QuickGELU via ACT Silu: x*sigmoid(1.702x) = silu(1.702x)/1.702 with the
    1/1.702 folded into fc2 weights and the 1.702 into the ACT input scale.
"""

import os
import numpy as np
import ml_dtypes

D = 768
H = 12
HD = 64
S = 77
FF = 3072
EPS = 1e-5
N_CORES = 8
B_FULL = 512
BPC = B_FULL // N_CORES          # 64 sequences per core
T_CORE = BPC * S                 # 4928 tokens per core
G_SEQ = 4                        # sequences per superblock
SB = G_SEQ * S                   # 308 tokens per superblock


def build_program(T=T_CORE, G=G_SEQ, use_silu=True, stages="ABCDEF"):
    import concourse.bass as bass
    import concourse.bacc as bacc
    import concourse.mybir as mybir
    import concourse.tile as tile
    from concourse.masks import make_identity
    from contextlib import ExitStack

    f32 = mybir.dt.float32
    bf16 = mybir.dt.bfloat16
    AX = mybir.AxisListType
    OP = mybir.AluOpType
    AF = mybir.ActivationFunctionType

    SBLK = G * S
    NSB = T // SBLK
    assert NSB * SBLK == T
    # token chunks within a superblock
    chunks = []
    off = 0
    while off < SBLK:
        w = min(128, SBLK - off)
        chunks.append((off, w))
        off += w

    nc = bacc.Bacc("TRN2", target_bir_lowering=False)

    x_d = nc.declare_dram_parameter("x", [T, D], f32, isOutput=False)
    wq_d = nc.declare_dram_parameter("wqT", [D, D], bf16, isOutput=False)
    wk_d = nc.declare_dram_parameter("wkT", [D, D], bf16, isOutput=False)
    wv_d = nc.declare_dram_parameter("wvT", [D, D], bf16, isOutput=False)
    wo_d = nc.declare_dram_parameter("woT", [D, D], bf16, isOutput=False)
    wf1_d = nc.declare_dram_parameter("fc1T", [D, FF], bf16, isOutput=False)
    wf2_d = nc.declare_dram_parameter("fc2T", [FF, D], bf16, isOutput=False)
    qb_d = nc.declare_dram_parameter("qb", [D], f32, isOutput=False)
    kb_d = nc.declare_dram_parameter("kb", [D], f32, isOutput=False)
    vb_d = nc.declare_dram_parameter("vb", [D], f32, isOutput=False)
    ob_d = nc.declare_dram_parameter("ob", [D], f32, isOutput=False)
    f1b_d = nc.declare_dram_parameter("fc1b", [FF], f32, isOutput=False)
    f2b_d = nc.declare_dram_parameter("fc2b", [D], f32, isOutput=False)
    mask_d = nc.declare_dram_parameter("mask", [S, S], bf16, isOutput=False)
    out_d = nc.declare_dram_parameter("out", [T, D], f32, isOutput=True)

    with tile.TileContext(nc) as tc, ExitStack() as ctx:
        singles = ctx.enter_context(tc.tile_pool(name="singles", bufs=1))
        xpool = ctx.enter_context(tc.tile_pool(name="xpool", bufs=3))
        x2pool = ctx.enter_context(tc.tile_pool(name="x2pool", bufs=3))
        actpool = ctx.enter_context(tc.tile_pool(name="actpool", bufs=1))
        ffpool = ctx.enter_context(tc.tile_pool(name="ffpool", bufs=24))
        outpool = ctx.enter_context(tc.tile_pool(name="outpool", bufs=2))
        attnpool = ctx.enter_context(tc.tile_pool(name="attnpool", bufs=2))
        statpool = ctx.enter_context(tc.tile_pool(name="statpool", bufs=2))
        pspool = ctx.enter_context(tc.tile_pool(name="pspool", bufs=4, space="PSUM"))

        # ---- constants / weights ----
        wq_sb = singles.tile([128, D // 128, D], bf16)
        wk_sb = singles.tile([128, D // 128, D], bf16)
        wv_sb = singles.tile([128, D // 128, D], bf16)
        wo_sb = singles.tile([128, D // 128, D], bf16)
        wf1_sb = singles.tile([128, D // 128, FF], bf16)
        wf2_sb = singles.tile([128, FF // 128, D], bf16)
        for sb_t, dr in ((wq_sb, wq_d), (wk_sb, wk_d), (wv_sb, wv_d),
                         (wo_sb, wo_d), (wf1_sb, wf1_d), (wf2_sb, wf2_d)):
            nc.sync.dma_start(out=sb_t, in_=dr[:].rearrange("(c p) o -> p c o", p=128))

        qb_sb = singles.tile([128, D // 128], f32)
        kb_sb = singles.tile([128, D // 128], f32)
        f1b_sb = singles.tile([128, FF // 128], f32)
        for sb_t, dr in ((qb_sb, qb_d), (kb_sb, kb_d), (f1b_sb, f1b_d)):
            nc.sync.dma_start(out=sb_t, in_=dr[:].rearrange("(c p) -> p c", p=128))

        # free-axis biases broadcast to all 128 partitions
        ob_bc = singles.tile([128, D], f32)
        f2b_bc = singles.tile([128, D], f32)
        vb_bc = singles.tile([128, D], f32)
        for sb_t, dr in ((ob_bc, ob_d), (f2b_bc, f2b_d), (vb_bc, vb_d)):
            src = bass.AP(tensor=dr[:].tensor, offset=dr[:].offset,
                          ap=[[0, 128]] + list(dr[:].ap))
            nc.sync.dma_start(out=sb_t, in_=src)

        mask_sb = singles.tile([S, S], bf16)
        nc.sync.dma_start(out=mask_sb, in_=mask_d[:])

        ident = singles.tile([128, 128], bf16)
        make_identity(nc, ident)

        eps_sb = singles.tile([128, 1], f32)
        nc.vector.memset(eps_sb, EPS)

        NCH = D // 128    # 6
        NFF = FF // 128   # 24

        def ln_normalize(src_tile, w, tag, bufs=2):
            """token-major [W, 768] fp32 -> normalized bf16 htok tile."""
            stats = statpool.tile([128, 3, 6], f32, tag=f"stats{tag}", name=f"stats{tag}")
            mv = statpool.tile([128, 2], f32, tag=f"mv{tag}", name=f"mv{tag}")
            xg = src_tile[:w].rearrange("p (s f) -> p s f", f=256)
            for i in range(3):
                nc.vector.bn_stats(out=stats[:w, i, :], in_=xg[:, i, :])
            nc.vector.bn_aggr(out=mv[:w], in_=stats[:w])
            mean = mv[:w, 0:1]
            rstd = mv[:w, 1:2]
            nc.scalar.activation(out=rstd, in_=rstd, func=AF.Sqrt,
                                 bias=eps_sb[:w], scale=1.0)
            nc.vector.reciprocal(out=rstd, in_=rstd)
            htok = statpool.tile([128, D], bf16, tag=f"htok{tag}", name=f"htok{tag}",
                                 bufs=bufs)
            nc.vector.tensor_scalar(out=htok[:w], in0=src_tile[:w],
                                    scalar1=mean, scalar2=rstd,
                                    op0=OP.subtract, op1=OP.mult)
            return htok

        def ln_transpose(htok, coff, w, hT, tag):
            for c in range(NCH):
                ps = pspool.tile([128, 128], bf16, tag="tr", name=f"trp{tag}")
                nc.tensor.transpose(ps[:, :w], htok[:w, c * 128:(c + 1) * 128],
                                    ident[:w, :w])
                nc.any.tensor_copy(out=hT[c][:, coff:coff + w], in_=ps[:, :w])

        def layer_norm_and_transpose(src_tile, coff, w, hT, tag):
            htok = ln_normalize(src_tile, w, tag)
            ln_transpose(htok, coff, w, hT, tag)

        def stage_A(isb):
            """load x, LN1 -> hT feature-major bf16; also x_tok += ob (gpsimd)."""
            t0 = isb * SBLK
            hT = [actpool.tile([128, SBLK], bf16, tag=f"hT{c}", name=f"hT{c}", bufs=2)
                  for c in range(NCH)]
            x_tiles = []
            for (coff, w) in chunks:
                x_tok = xpool.tile([128, D], f32, tag="xtok", name="xtok")
                nc.sync.dma_start(out=x_tok[:w], in_=x_d[t0 + coff: t0 + coff + w, :])
                x_tiles.append(x_tok)
                layer_norm_and_transpose(x_tok, coff, w, hT, "A")
            return hT, x_tiles

        def stage_D_chunk(ci, ctxT, x_tiles, x2_tiles):
            coff, w = chunks[ci]
            x2 = x2pool.tile([128, D], f32, tag="x2tok", name="x2tok")
            for half in range(2):
                ps = pspool.tile([128, 384], f32, tag="big", name="pso")
                for d in range(NCH):
                    nc.tensor.matmul(ps[:w], lhsT=ctxT[d][:, coff:coff + w],
                                     rhs=wo_sb[:, d, half * 384:(half + 1) * 384],
                                     start=(d == 0), stop=(d == NCH - 1))
                sl = slice(half * 384, (half + 1) * 384)
                nc.vector.tensor_tensor(out=x2[:w, sl], in0=ps[:w],
                                        in1=ob_bc[:w, sl], op=OP.add)
                nc.vector.tensor_tensor(out=x2[:w, sl], in0=x2[:w, sl],
                                        in1=x_tiles[ci][:w, sl], op=OP.add)
            x2_tiles.append(x2)

        cur = stage_A(0)
        for isb in range(NSB):
            t0 = isb * SBLK
            hT, x_tiles = cur

            # ---- stage B: q/k projections (feature-major, bf16) ----
            qT = [actpool.tile([128, SBLK], bf16, tag=f"qT{c}", name=f"qT{c}", bufs=2)
                  for c in range(NCH)]
            kT = [actpool.tile([128, SBLK], bf16, tag=f"kT{c}", name=f"kT{c}", bufs=2)
                  for c in range(NCH)]
            for dst, w_sb, b_sb in ((qT, wq_sb, qb_sb), (kT, wk_sb, kb_sb)):
                for c in range(NCH):
                    ps = pspool.tile([128, SBLK], f32, tag="big", name="psqkv")
                    for d in range(NCH):
                        nc.tensor.matmul(ps, lhsT=w_sb[:, d, c * 128:(c + 1) * 128],
                                         rhs=hT[d], start=(d == 0), stop=(d == NCH - 1))
                    nc.vector.tensor_scalar(out=dst[c], in0=ps,
                                            scalar1=b_sb[:, c:c + 1], scalar2=None,
                                            op0=OP.add)

            # ---- stage C: attention per sequence, with D/E chunks emitted as
            #      soon as the sequences covering them are done ----
            ctxT = [actpool.tile([128, SBLK], bf16, tag=f"ctxT{c}", name=f"ctxT{c}",
                                 bufs=2)
                    for c in range(NCH)]
            h2T = [actpool.tile([128, SBLK], bf16, tag=f"h2T{c}", name=f"h2T{c}")
                   for c in range(NCH)]
            x2_tiles = []
            h2toks = []
            next_chunk = 0
            for s in range(G):
                so = s * S
                # v for this sequence, token-major directly (swapped operands)
                vtok = attnpool.tile([S, H, HD], bf16, tag="vtok", name="vtok")
                for half in range(2):
                    psv = pspool.tile([S, 384], f32, tag="big", name="psvtok")
                    for d in range(NCH):
                        nc.tensor.matmul(psv,
                                         lhsT=hT[d][:, so:so + S],
                                         rhs=wv_sb[:, d, half * 384:(half + 1) * 384],
                                         start=(d == 0), stop=(d == NCH - 1))
                    nc.vector.tensor_tensor(
                        out=vtok[:, half * 6:(half + 1) * 6, :], in0=psv,
                        in1=vb_bc[:S, half * 384:(half + 1) * 384], op=OP.add)
                p_sb = attnpool.tile([S, H, S], bf16, tag="p", name="p_sb", bufs=1)
                denom = statpool.tile([S, H], f32, tag="denom", name="denom")
                attnT = attnpool.tile([S, H, S], bf16, tag="attnT", name="attnT")
                for half in range(2):
                    hh = half * 6
                    for i in range(6):
                        h = hh + i
                        c, po = h // 2, 64 * (h % 2)
                        ps = pspool.tile([S, S], f32, tag="tr", name="pssc")
                        nc.tensor.matmul(ps,
                                         lhsT=qT[c][po:po + 64, so:so + S],
                                         rhs=kT[c][po:po + 64, so:so + S],
                                         start=True, stop=True)
                        nc.scalar.activation(out=p_sb[:, h, :], in_=ps, func=AF.Exp)
                    nc.vector.tensor_tensor(
                        out=p_sb[:, hh:hh + 6, :], in0=p_sb[:, hh:hh + 6, :],
                        in1=mask_sb[:, None, :].to_broadcast((S, 6, S)), op=OP.mult)
                    nc.vector.reduce_sum(out=denom[:, hh:hh + 6],
                                         in_=p_sb[:, hh:hh + 6, :], axis=AX.X)
                    nc.vector.reciprocal(out=denom[:, hh:hh + 6],
                                         in_=denom[:, hh:hh + 6])
                    nc.vector.tensor_tensor(
                        out=p_sb[:, hh:hh + 6, :], in0=p_sb[:, hh:hh + 6, :],
                        in1=denom[:, hh:hh + 6, None].to_broadcast((S, 6, S)),
                        op=OP.mult)
                    for i in range(6):
                        h = hh + i
                        psa = pspool.tile([S, S + 1], bf16, tag="tr", name="psattnT")
                        nc.tensor.transpose(psa[:, :S], p_sb[:, h, :], ident[:S, :S])
                        nc.any.tensor_copy(out=attnT[:, h, :], in_=psa[:, :S])
                # ctx per head -> ctxT chunks: head 2c -> parts 0:64, 2c+1 -> 64:128
                for h in range(H):
                    c, po = h // 2, 64 * (h % 2)
                    psc = pspool.tile([64, S], f32, tag="tr", name="psctx")
                    nc.tensor.matmul(psc, lhsT=vtok[:, h, :], rhs=attnT[:, h, :],
                                     start=True, stop=True)
                    nc.vector.tensor_copy(out=ctxT[c][po:po + 64, so:so + S],
                                          in_=psc)
                # emit O-proj + residual + LN2 for chunks fully covered
                done_tokens = (s + 1) * S
                while (next_chunk < len(chunks)
                       and chunks[next_chunk][0] + chunks[next_chunk][1]
                       <= done_tokens):
                    ci = next_chunk
                    stage_D_chunk(ci, ctxT, x_tiles, x2_tiles)
                    coff, w = chunks[ci]
                    h2toks.append(ln_normalize(x2_tiles[ci], w, "E", bufs=3))
                    next_chunk += 1

            # E transposes (emitted after C so the in-order PE isn't blocked
            # mid-attention waiting on the LN chains)
            for ci, (coff, w) in enumerate(chunks):
                ln_transpose(h2toks[ci], coff, w, h2T, "E")

            # ---- stage F: MLP ----
            ff1 = []
            for f in range(NFF):
                ps = pspool.tile([128, SBLK], f32, tag="big", name="psff")
                for d in range(NCH):
                    nc.tensor.matmul(ps, lhsT=wf1_sb[:, d, f * 128:(f + 1) * 128],
                                     rhs=h2T[d], start=(d == 0), stop=(d == NCH - 1))
                f1 = ffpool.tile([128, SBLK], bf16, tag="ff1", name="ff1")
                if use_silu:
                    # f1 = silu(1.702*ps + 1.702*b) = 1.702*quickgelu(ps+b);
                    # the 1/1.702 is folded into fc2T host-side.
                    nc.scalar.activation(out=f1, in_=ps, func=AF.Silu,
                                         bias=f1b_sb[:, f:f + 1], scale=1.702)
                else:
                    # CoreSim fallback: sigmoid + 2 DVE ops, same contract
                    sgt = statpool.tile([128, SBLK], bf16, tag="sgt", name="sgt")
                    nc.scalar.activation(out=sgt, in_=ps, func=AF.Sigmoid,
                                         bias=f1b_sb[:, f:f + 1], scale=1.702)
                    at = statpool.tile([128, SBLK], f32, tag="at", name="at")
                    nc.vector.tensor_scalar(out=at, in0=ps, scalar1=1.702,
                                            scalar2=f1b_sb[:, f:f + 1],
                                            op0=OP.mult, op1=OP.add)
                    nc.vector.tensor_tensor(out=f1, in0=at, in1=sgt, op=OP.mult)
                ff1.append(f1)
            for ci, (coff, w) in enumerate(chunks):
                pss = [pspool.tile([128, 384], f32, tag="big", name="psf2")
                       for _ in range(2)]
                for f in range(NFF):
                    for half in range(2):
                        nc.tensor.matmul(pss[half][:w],
                                         lhsT=ff1[f][:, coff:coff + w],
                                         rhs=wf2_sb[:, f, half * 384:(half + 1) * 384],
                                         start=(f == 0), stop=(f == NFF - 1),
                                         skip_group_check=True)
                o_tok = outpool.tile([128, D], f32, tag="otok", name="otok")
                for half in range(2):
                    sl = slice(half * 384, (half + 1) * 384)
                    nc.vector.tensor_tensor(out=o_tok[:w, sl], in0=pss[half][:w],
                                            in1=f2b_bc[:w, sl], op=OP.add)
                    nc.vector.tensor_tensor(out=o_tok[:w, sl], in0=o_tok[:w, sl],
                                            in1=x2_tiles[ci][:w, sl], op=OP.add)
                nc.sync.dma_start(out=out_d[t0 + coff: t0 + coff + w, :],
                                  in_=o_tok[:w])

            # ---- prefetch next superblock's stage A (overlaps F on DVE) ----
            if isb + 1 < NSB:
                cur = stage_A(isb + 1)

    nc.compile()
    return nc


def prep_shared(inputs):
    """Fold LN affine params / scale constants into weights -> shared in_map entries."""
    bf = ml_dtypes.bfloat16
    f32 = np.float32
    g = {k: np.asarray(v, dtype=np.float32) for k, v in inputs.items() if k != "x"}

    wqT = (g["ln1_w"][:, None] * g["qw"].T * 0.125).astype(bf)
    wkT = (g["ln1_w"][:, None] * g["kw"].T).astype(bf)
    wvT = (g["ln1_w"][:, None] * g["vw"].T).astype(bf)
    woT = np.ascontiguousarray(g["ow"].T).astype(bf)
    fc1T = (g["ln2_w"][:, None] * g["fc1_w"].T).astype(bf)
    fc2T = (g["fc2_w"].T / 1.702).astype(bf)

    qb = ((g["ln1_b"] @ g["qw"].T + g["qb"]) * 0.125).astype(f32)
    kb = (g["ln1_b"] @ g["kw"].T + g["kb"]).astype(f32)
    vb = (g["ln1_b"] @ g["vw"].T + g["vb"]).astype(f32)
    ob = g["ob"].astype(f32)
    fc1b = ((g["ln2_b"] @ g["fc1_w"].T + g["fc1_b"]) * 1.702).astype(f32)
    fc2b = g["fc2_b"].astype(f32)

    mask = np.tril(np.ones((S, S), np.float32)).astype(bf)   # [q, k], k<=q allowed

    return dict(wqT=wqT, wkT=wkT, wvT=wvT, woT=woT, fc1T=fc1T, fc2T=fc2T,
                qb=qb, kb=kb, vb=vb, ob=ob, fc1b=fc1b, fc2b=fc2b, mask=mask)


def prep_host_inputs(inputs):
    shared = prep_shared(inputs)
    x = np.asarray(inputs["x"], dtype=np.float32)
    in_maps = []
    for c in range(N_CORES):
        xc = np.ascontiguousarray(
            x[c * BPC:(c + 1) * BPC].reshape(T_CORE, D).astype(np.float32))
        in_maps.append(dict(shared, x=xc))
    return in_maps


_CACHED_NC = None


def _get_nc():
    global _CACHED_NC
    if _CACHED_NC is None:
        _CACHED_NC = build_program()
    return _CACHED_NC


def run(inputs, trace=False):
    from concourse.bass_utils import run_bass_kernel_spmd
    nc = _get_nc()
    in_maps = prep_host_inputs(inputs)
    res = run_bass_kernel_spmd(nc, in_maps, list(range(N_CORES)), trace=trace)
    outs = [np.asarray(res.results[c]["out"], dtype=np.float32).reshape(BPC, S, D)
            for c in range(N_CORES)]
    full = np.concatenate(outs, axis=0)
    return full, res


def kernel(**inputs):
    full, _ = run(inputs, trace=False)
    return full



# revision 3
# speedup vs baseline: 1.0311x; 1.0311x over previous
"""CLIP encoder layer on 8 trn2 NeuronCores, pure data parallel over batch.

Layout strategy (per core, batch shard of 64 sequences = 4928 tokens):
  - x arrives token-major [T, 768] fp32.
  - LayerNorm runs token-major (tokens on partitions, bn_stats/bn_aggr),
    LN scale/bias folded into the downstream projection weights host-side.
    rstd computed as exp(-0.5*ln(var+eps)) so LN shares the scalar-engine
    natural_log_exp table set with attention's EXP (no sqrt-set thrash).
  - Normalized activations are PE-transposed (bf16) to feature-major
    [768, N] for the projections (weights stationary, activations moving).
  - Attention per sequence (S=77): scoresT[k,q] = kT.T @ qT per head,
    written directly in k-major orientation so no post-softmax transpose is
    needed; 6 even heads share one PSUM bank, 6 odd heads another (row-group
    packed pairs run concurrently in the PE).  The causal mask is ADDED via
    an identity-matmul accumulation (-1e5 above diagonal) before EXP, so the
    only cross-engine dependency between scores and ctx is a single EXP per
    bank.  Softmax denominators come out of a ones-matrix matmul
    (ones77.T @ pT -> every partition holds the per-query denominator);
    normalization is fused into the ctx PSUM->SBUF evacuation multiply.
  - ctx matmuls are column-packed pairs writing [128, 77] PSUM tiles that
    are already in the ctxT feature-major layout the O-projection wants.
  - O-projection runs with swapped operands (activations stationary) so its
    output comes out token-major for the residual add (x+ob precomputed on
    the otherwise-idle GpSimd engine) and the second LayerNorm.
  - FC1 feature-major (weights stationary).  FC2 also feature-major
    (44K vs 55K PE cycles), then PE-transposed back to token-major with the
    final residual add fused into the PSUM evacuation.
  - All matmuls in bf16 (fp32 PSUM accumulation); fp32 elsewhere.
    QuickGELU via ACT Silu: x*sigmoid(1.702x) = silu(1.702x)/1.702 with the
    1/1.702 folded into fc2 weights and the 1.702 into the ACT input scale.
"""

import os
import numpy as np
import ml_dtypes

D = 768
H = 12
HD = 64
S = 77
FF = 3072
EPS = 1e-5
N_CORES = 8
B_FULL = 512
BPC = B_FULL // N_CORES          # 64 sequences per core
T_CORE = BPC * S                 # 4928 tokens per core
G_SEQ = 4                        # sequences per superblock
SB = G_SEQ * S                   # 308 tokens per superblock
MASK_NEG = -1.0e5                # additive causal mask (exp -> exact 0)


def build_program(T=T_CORE, G=G_SEQ):
    import concourse.bass as bass
    import concourse.bacc as bacc
    import concourse.mybir as mybir
    import concourse.tile as tile
    from concourse.masks import make_identity
    from contextlib import ExitStack

    f32 = mybir.dt.float32
    bf16 = mybir.dt.bfloat16
    AX = mybir.AxisListType
    OP = mybir.AluOpType
    AF = mybir.ActivationFunctionType

    SBLK = G * S
    NSB = T // SBLK
    assert NSB * SBLK == T
    NH2 = H // 2                  # 6 head pairs
    SW = NH2 * S                  # 462 score columns per bank
    # token chunks within a superblock
    chunks = []
    off = 0
    while off < SBLK:
        w = min(128, SBLK - off)
        chunks.append((off, w))
        off += w

    nc = bacc.Bacc("TRN2", target_bir_lowering=False)

    x_d = nc.declare_dram_parameter("x", [T, D], f32, isOutput=False)
    wq_d = nc.declare_dram_parameter("wqT", [D, D], bf16, isOutput=False)
    wk_d = nc.declare_dram_parameter("wkT", [D, D], bf16, isOutput=False)
    wv_d = nc.declare_dram_parameter("wvT", [D, D], bf16, isOutput=False)
    wo_d = nc.declare_dram_parameter("woT", [D, D], bf16, isOutput=False)
    wf1_d = nc.declare_dram_parameter("fc1T", [D, FF], bf16, isOutput=False)
    wf2_d = nc.declare_dram_parameter("fc2T", [FF, D], bf16, isOutput=False)
    qb_d = nc.declare_dram_parameter("qb", [D], f32, isOutput=False)
    kb_d = nc.declare_dram_parameter("kb", [D], f32, isOutput=False)
    vb_d = nc.declare_dram_parameter("vb", [D], f32, isOutput=False)
    ob_d = nc.declare_dram_parameter("ob", [D], f32, isOutput=False)
    f1b_d = nc.declare_dram_parameter("fc1b", [FF], f32, isOutput=False)
    f2b_d = nc.declare_dram_parameter("fc2b", [D], f32, isOutput=False)
    mask_d = nc.declare_dram_parameter("mask6", [S, SW], bf16, isOutput=False)
    out_d = nc.declare_dram_parameter("out", [T, D], f32, isOutput=True)

    with tile.TileContext(nc) as tc, ExitStack() as ctx:
        singles = ctx.enter_context(tc.tile_pool(name="singles", bufs=1))
        xpool = ctx.enter_context(tc.tile_pool(name="xpool", bufs=3))
        x2pool = ctx.enter_context(tc.tile_pool(name="x2pool", bufs=3))
        actpool = ctx.enter_context(tc.tile_pool(name="actpool", bufs=1))
        ffpool = ctx.enter_context(tc.tile_pool(name="ffpool", bufs=24))
        outpool = ctx.enter_context(tc.tile_pool(name="outpool", bufs=1))
        attnpool = ctx.enter_context(tc.tile_pool(name="attnpool", bufs=2))
        statpool = ctx.enter_context(tc.tile_pool(name="statpool", bufs=2))
        pspool = ctx.enter_context(tc.tile_pool(name="pspool", bufs=2, space="PSUM"))

        # ---- constants / weights ----
        wq_sb = singles.tile([128, D // 128, D], bf16)
        wk_sb = singles.tile([128, D // 128, D], bf16)
        wv_sb = singles.tile([128, D // 128, D], bf16)
        wo_sb = singles.tile([128, D // 128, D], bf16)
        wf1_sb = singles.tile([128, D // 128, FF], bf16)
        wf2_sb = singles.tile([128, FF // 128, D], bf16)
        for sb_t, dr in ((wq_sb, wq_d), (wk_sb, wk_d), (wv_sb, wv_d),
                         (wo_sb, wo_d), (wf1_sb, wf1_d), (wf2_sb, wf2_d)):
            nc.sync.dma_start(out=sb_t, in_=dr[:].rearrange("(c p) o -> p c o", p=128))

        qb_sb = singles.tile([128, D // 128], f32)
        kb_sb = singles.tile([128, D // 128], f32)
        f1b_sb = singles.tile([128, FF // 128], f32)
        f2b_sb = singles.tile([128, D // 128], f32)
        for sb_t, dr in ((qb_sb, qb_d), (kb_sb, kb_d), (f1b_sb, f1b_d),
                         (f2b_sb, f2b_d)):
            nc.sync.dma_start(out=sb_t, in_=dr[:].rearrange("(c p) -> p c", p=128))

        # free-axis biases broadcast to all 128 partitions
        ob_bc = singles.tile([128, D], f32)
        vb_bc = singles.tile([128, D], f32)
        for sb_t, dr in ((ob_bc, ob_d), (vb_bc, vb_d)):
            src = bass.AP(tensor=dr[:].tensor, offset=dr[:].offset,
                          ap=[[0, 128]] + list(dr[:].ap))
            nc.sync.dma_start(out=sb_t, in_=src)

        mask6_sb = singles.tile([S, SW], bf16)
        nc.sync.dma_start(out=mask6_sb, in_=mask_d[:])

        ident = singles.tile([128, 128], bf16)
        make_identity(nc, ident)

        ones77 = singles.tile([S, 128], bf16)
        nc.vector.memset(ones77, 1.0)

        eps_sb = singles.tile([128, 1], f32)
        nc.vector.memset(eps_sb, EPS)

        NCH = D // 128    # 6
        NFF = FF // 128   # 24

        def ln_normalize(src_tile, w, tag, bufs=2):
            """token-major [w, 768] fp32 -> normalized bf16 htok tile."""
            stats = statpool.tile([128, 3, 6], f32, tag=f"stats{tag}", name=f"stats{tag}")
            mv = statpool.tile([128, 3], f32, tag=f"mv{tag}", name=f"mv{tag}")
            xg = src_tile[:w].rearrange("p (s f) -> p s f", f=256)
            for i in range(3):
                nc.vector.bn_stats(out=stats[:w, i, :], in_=xg[:, i, :])
            nc.vector.bn_aggr(out=mv[:w, 0:2], in_=stats[:w])
            mean = mv[:w, 0:1]
            var = mv[:w, 1:2]
            lnv = mv[:w, 2:3]
            # rstd = exp(-0.5*ln(var+eps)) - stays in the natural_log_exp
            # activation-table set shared with attention's EXP.
            nc.scalar.activation(out=lnv, in_=var, func=AF.Ln,
                                 bias=eps_sb[:w], scale=1.0)
            nc.scalar.activation(out=var, in_=lnv, func=AF.Exp,
                                 bias=0.0, scale=-0.5)
            rstd = var
            htok = statpool.tile([128, D], bf16, tag=f"htok{tag}", name=f"htok{tag}",
                                 bufs=bufs)
            nc.vector.tensor_scalar(out=htok[:w], in0=src_tile[:w],
                                    scalar1=mean, scalar2=rstd,
                                    op0=OP.subtract, op1=OP.mult)
            return htok

        def ln_transpose(htok, coff, w, hT, tag):
            for c in range(NCH):
                ps = pspool.tile([128, 128], bf16, tag="tr", name=f"trp{tag}")
                nc.tensor.transpose(ps[:, :w], htok[:w, c * 128:(c + 1) * 128],
                                    ident[:w, :w])
                nc.any.tensor_copy(out=hT[c][:, coff:coff + w], in_=ps[:, :w])

        def stage_A(isb):
            """load x, LN1 -> hT feature-major bf16; then x_tok += ob (gpsimd)."""
            t0 = isb * SBLK
            hT = [actpool.tile([128, SBLK], bf16, tag=f"hT{c}", name=f"hT{c}", bufs=2)
                  for c in range(NCH)]
            x_tiles = []
            for (coff, w) in chunks:
                x_tok = xpool.tile([128, D], f32, tag="xtok", name="xtok")
                nc.sync.dma_start(out=x_tok[:w], in_=x_d[t0 + coff: t0 + coff + w, :])
                x_tiles.append(x_tok)
                htok = ln_normalize(x_tok, w, "A")
                ln_transpose(htok, coff, w, hT, "A")
                # after LN consumed raw x: fold the o-proj bias into the
                # residual in place on the idle GpSimd engine.
                nc.gpsimd.tensor_tensor(out=x_tok[:w], in0=x_tok[:w],
                                        in1=ob_bc[:w], op=OP.add)
            return hT, x_tiles

        def stage_D_chunk(ci, ctxT, x_tiles, x2_tiles):
            coff, w = chunks[ci]
            x2 = x2pool.tile([128, D], f32, tag="x2tok", name="x2tok")
            for half in range(2):
                ps = pspool.tile([128, 384], f32, tag="big", name="pso")
                for d in range(NCH):
                    nc.tensor.matmul(ps[:w], lhsT=ctxT[d][:, coff:coff + w],
                                     rhs=wo_sb[:, d, half * 384:(half + 1) * 384],
                                     start=(d == 0), stop=(d == NCH - 1))
                sl = slice(half * 384, (half + 1) * 384)
                nc.vector.tensor_tensor(out=x2[:w, sl], in0=ps[:w],
                                        in1=x_tiles[ci][:w, sl], op=OP.add)
            x2_tiles.append(x2)

        cur = stage_A(0)
        for isb in range(NSB):
            t0 = isb * SBLK
            hT, x_tiles = cur

            # ---- stage B: q/k projections (feature-major, bf16) ----
            qT = [actpool.tile([128, SBLK], bf16, tag=f"qT{c}", name=f"qT{c}")
                  for c in range(NCH)]
            kT = [actpool.tile([128, SBLK], bf16, tag=f"kT{c}", name=f"kT{c}")
                  for c in range(NCH)]
            for dst, w_sb, b_sb in ((qT, wq_sb, qb_sb), (kT, wk_sb, kb_sb)):
                for c in range(NCH):
                    ps = pspool.tile([128, SBLK], f32, tag="big", name="psqkv")
                    for d in range(NCH):
                        nc.tensor.matmul(ps, lhsT=w_sb[:, d, c * 128:(c + 1) * 128],
                                         rhs=hT[d], start=(d == 0), stop=(d == NCH - 1))
                    nc.vector.tensor_scalar(out=dst[c], in0=ps,
                                            scalar1=b_sb[:, c:c + 1], scalar2=None,
                                            op0=OP.add)

            # ---- stage C: attention per sequence ----
            ctxT = [actpool.tile([128, SBLK], bf16, tag=f"ctxT{c}", name=f"ctxT{c}",
                                 bufs=2)
                    for c in range(NCH)]
            h2T = [actpool.tile([128, SBLK], bf16, tag=f"h2T{c}", name=f"h2T{c}")
                   for c in range(NCH)]
            x2_tiles = []
            h2toks = []
            next_chunk = 0
            for s in range(G):
                so = s * S
                # v for this sequence, token-major directly (swapped operands)
                vtok = attnpool.tile([S, H, HD], bf16, tag="vtok", name="vtok")
                for half in range(2):
                    psv = pspool.tile([S, 384], f32, tag="big", name="psvtok")
                    for d in range(NCH):
                        nc.tensor.matmul(psv,
                                         lhsT=hT[d][:, so:so + S],
                                         rhs=wv_sb[:, d, half * 384:(half + 1) * 384],
                                         start=(d == 0), stop=(d == NCH - 1))
                    nc.vector.tensor_tensor(
                        out=vtok[:, half * 6:(half + 1) * 6, :], in0=psv,
                        in1=vb_bc[:S, half * 384:(half + 1) * 384], op=OP.add)
                # scoresT[k, q] per head; even heads -> bank A, odd -> bank B.
                # Row-group packing: even heads live on partitions 0:64 of
                # their qT/kT chunk, odd heads on 64:128 -> pairs overlap.
                psc = [pspool.tile([S, SW], f32, tag="sc", name=f"psc{a}")
                       for a in range(2)]
                for j in range(NH2):
                    for a in range(2):
                        h = 2 * j + a
                        c, po = h // 2, 64 * (h % 2)
                        nc.tensor.matmul(psc[a][:, j * S:(j + 1) * S],
                                         lhsT=kT[c][po:po + 64, so:so + S],
                                         rhs=qT[c][po:po + 64, so:so + S],
                                         start=(j == 0), stop=False,
                                         skip_group_check=True)
                # additive causal mask via identity-matmul accumulation
                pT = attnpool.tile([S, 2, SW], bf16, tag="pT", name="pT")
                for a in range(2):
                    nc.tensor.matmul(psc[a], lhsT=ident[:S, :S], rhs=mask6_sb,
                                     start=False, stop=True, skip_group_check=True)
                    nc.scalar.activation(out=pT[:, a, :], in_=psc[a], func=AF.Exp)
                # denominators broadcast to all partitions: ones77.T @ pT
                dben = [pspool.tile([128, SW], f32, tag="sc", name=f"dben{a}")
                        for a in range(2)]
                rp = attnpool.tile([128, SW], bf16, tag="rp", name="rp")
                for a in range(2):
                    nc.tensor.matmul(dben[a], lhsT=ones77, rhs=pT[:, a, :],
                                     start=True, stop=True)
                with nc.allow_low_precision(reason="softmax recip in bf16"):
                    nc.vector.reciprocal(out=rp[0:64, :], in_=dben[0][0:64, :])
                    nc.vector.reciprocal(out=rp[64:128, :], in_=dben[1][64:128, :])
                # ctx per head pair, column-packed into [128, 77] PSUM already
                # in ctxT layout; normalization fused into the evacuation.
                for j in range(NH2):
                    ctxp = pspool.tile([128, S], f32, tag="ctxp", name="ctxp")
                    for a in range(2):
                        h = 2 * j + a
                        nc.tensor.matmul(ctxp[64 * a:64 * a + 64, :],
                                         lhsT=vtok[:, h, :],
                                         rhs=pT[:, a, j * S:(j + 1) * S],
                                         start=True, stop=True,
                                         skip_group_check=True)
                    nc.vector.tensor_tensor(out=ctxT[j][:, so:so + S], in0=ctxp,
                                            in1=rp[:, j * S:(j + 1) * S],
                                            op=OP.mult)
                # emit O-proj + residual + LN2 for chunks fully covered
                done_tokens = (s + 1) * S
                while (next_chunk < len(chunks)
                       and chunks[next_chunk][0] + chunks[next_chunk][1]
                       <= done_tokens):
                    ci = next_chunk
                    stage_D_chunk(ci, ctxT, x_tiles, x2_tiles)
                    coff, w = chunks[ci]
                    h2toks.append(ln_normalize(x2_tiles[ci], w, "E", bufs=3))
                    next_chunk += 1

            # E transposes (emitted after C so the in-order PE isn't blocked
            # mid-attention waiting on the LN chains)
            for ci, (coff, w) in enumerate(chunks):
                ln_transpose(h2toks[ci], coff, w, h2T, "E")

            # ---- stage F: MLP ----
            ff1 = []
            for f in range(NFF):
                ps = pspool.tile([128, SBLK], f32, tag="big", name="psff")
                for d in range(NCH):
                    nc.tensor.matmul(ps, lhsT=wf1_sb[:, d, f * 128:(f + 1) * 128],
                                     rhs=h2T[d], start=(d == 0), stop=(d == NCH - 1))
                f1 = ffpool.tile([128, SBLK], bf16, tag="ff1", name="ff1")
                # f1 = silu(1.702*ps + 1.702*b) = 1.702*quickgelu(ps+b);
                # the 1/1.702 is folded into fc2T host-side.
                nc.scalar.activation(out=f1, in_=ps, func=AF.Silu,
                                     bias=f1b_sb[:, f:f + 1], scale=1.702)
                ff1.append(f1)
            # FC2 feature-major (weights stationary, tokens streaming), then
            # PE-transpose each dout chunk back to token-major with the final
            # residual fused into the PSUM evacuation add.
            o_toks = [outpool.tile([128, D], f32, tag=f"otok{ci}", name=f"otok{ci}")
                      for ci in range(len(chunks))]
            for c in range(NCH):
                ps = pspool.tile([128, SBLK], f32, tag="big", name="psf2")
                for f in range(NFF):
                    nc.tensor.matmul(ps, lhsT=wf2_sb[:, f, c * 128:(c + 1) * 128],
                                     rhs=ff1[f], start=(f == 0), stop=(f == NFF - 1))
                x3c = statpool.tile([128, SBLK], bf16, tag="x3", name="x3")
                nc.vector.tensor_scalar(out=x3c, in0=ps,
                                        scalar1=f2b_sb[:, c:c + 1], scalar2=None,
                                        op0=OP.add)
                for ci, (coff, w) in enumerate(chunks):
                    tr = pspool.tile([128, 128], bf16, tag="tr", name="trf")
                    nc.tensor.transpose(tr[:w, :], x3c[:, coff:coff + w], ident)
                    nc.vector.tensor_tensor(
                        out=o_toks[ci][:w, c * 128:(c + 1) * 128],
                        in0=tr[:w, :],
                        in1=x2_tiles[ci][:w, c * 128:(c + 1) * 128], op=OP.add)
            for ci, (coff, w) in enumerate(chunks):
                nc.sync.dma_start(out=out_d[t0 + coff: t0 + coff + w, :],
                                  in_=o_toks[ci][:w])

            # ---- prefetch next superblock's stage A (overlaps F on DVE) ----
            if isb + 1 < NSB:
                cur = stage_A(isb + 1)

    nc.compile()
    return nc


def prep_shared(inputs):
    """Fold LN affine params / scale constants into weights -> shared in_map entries."""
    bf = ml_dtypes.bfloat16
    f32 = np.float32
    g = {k: np.asarray(v, dtype=np.float32) for k, v in inputs.items() if k != "x"}

    wqT = (g["ln1_w"][:, None] * g["qw"].T * 0.125).astype(bf)
    wkT = (g["ln1_w"][:, None] * g["kw"].T).astype(bf)
    wvT = (g["ln1_w"][:, None] * g["vw"].T).astype(bf)
    woT = np.ascontiguousarray(g["ow"].T).astype(bf)
    fc1T = (g["ln2_w"][:, None] * g["fc1_w"].T).astype(bf)
    fc2T = (g["fc2_w"].T / 1.702).astype(bf)

    qb = ((g["ln1_b"] @ g["qw"].T + g["qb"]) * 0.125).astype(f32)
    kb = (g["ln1_b"] @ g["kw"].T + g["kb"]).astype(f32)
    vb = (g["ln1_b"] @ g["vw"].T + g["vb"]).astype(f32)
    ob = g["ob"].astype(f32)
    fc1b = ((g["ln2_b"] @ g["fc1_w"].T + g["fc1_b"]) * 1.702).astype(f32)
    fc2b = g["fc2_b"].astype(f32)

    # additive causal mask in scoresT[k, q] orientation (k > q masked),
    # tiled 6x along q for the per-bank [77, 462] accumulation matmul.
    m1 = np.where(np.arange(S)[:, None] > np.arange(S)[None, :], MASK_NEG, 0.0)
    mask6 = np.tile(m1.astype(np.float32), (1, H // 2)).astype(bf)

    return dict(wqT=wqT, wkT=wkT, wvT=wvT, woT=woT, fc1T=fc1T, fc2T=fc2T,
                qb=qb, kb=kb, vb=vb, ob=ob, fc1b=fc1b, fc2b=fc2b, mask6=mask6)


def prep_host_inputs(inputs):
    shared = prep_shared(inputs)
    x = np.asarray(inputs["x"], dtype=np.float32)
    in_maps = []
    for c in range(N_CORES):
        xc = np.ascontiguousarray(
            x[c * BPC:(c + 1) * BPC].reshape(T_CORE, D).astype(np.float32))
        in_maps.append(dict(shared, x=xc))
    return in_maps


_CACHED_NC = None


def _get_nc():
    global _CACHED_NC
    if _CACHED_NC is None:
        _CACHED_NC = build_program()
    return _CACHED_NC


def run(inputs, trace=False):
    from concourse.bass_utils import run_bass_kernel_spmd
    nc = _get_nc()
    in_maps = prep_host_inputs(inputs)
    res = run_bass_kernel_spmd(nc, in_maps, list(range(N_CORES)), trace=trace)
    outs = [np.asarray(res.results[c]["out"], dtype=np.float32).reshape(BPC, S, D)
            for c in range(N_CORES)]
    full = np.concatenate(outs, axis=0)
    return full, res


def kernel(**inputs):
    full, _ = run(inputs, trace=False)
    return full


# revision 6
# speedup vs baseline: 1.2167x; 1.1800x over previous
"""CLIP encoder layer on 8 trn2 NeuronCores, pure data parallel over batch.

Layout strategy (per core, batch shard of 64 sequences = 4928 tokens):
  - x arrives token-major [T, 768] fp32.
  - LayerNorm runs token-major (tokens on partitions, bn_stats/bn_aggr),
    LN scale/bias folded into the downstream projection weights host-side.
    rstd computed as exp(-0.5*ln(var+eps)) so LN shares the scalar-engine
    natural_log_exp table set with attention's EXP (no sqrt-set thrash).
  - Normalized activations are PE-transposed (bf16) to feature-major
    [768, N] for the projections (weights stationary, activations moving).
  - Attention per sequence (S=77): scoresT[k,q] = kT.T @ qT per head,
    written directly in k-major orientation so no post-softmax transpose is
    needed; 6 even heads share one PSUM bank, 6 odd heads another (row-group
    packed pairs run concurrently in the PE).  The causal mask is ADDED via
    an identity-matmul accumulation (-1e5 above diagonal) before EXP, so the
    only cross-engine dependency between scores and ctx is a single EXP per
    bank.  Softmax denominators come out of a ones-matrix matmul
    (ones77.T @ pT -> every partition holds the per-query denominator);
    normalization is fused into the ctx PSUM->SBUF evacuation multiply.
  - ctx matmuls are column-packed pairs writing [128, 77] PSUM tiles that
    are already in the ctxT feature-major layout the O-projection wants.
  - O-projection runs with swapped operands (activations stationary) so its
    output comes out token-major for the residual add (x+ob precomputed on
    the otherwise-idle GpSimd engine) and the second LayerNorm.
  - FC1 feature-major (weights stationary).  FC2 also feature-major
    (44K vs 55K PE cycles), then PE-transposed back to token-major with the
    final residual add fused into the PSUM evacuation.
  - All matmuls in bf16 (fp32 PSUM accumulation); fp32 elsewhere.
    QuickGELU via ACT Silu: x*sigmoid(1.702x) = silu(1.702x)/1.702 with the
    1/1.702 folded into fc2 weights and the 1.702 into the ACT input scale.
"""

import os
import numpy as np
import ml_dtypes

D = 768
H = 12
HD = 64
S = 77
FF = 3072
EPS = 1e-5
N_CORES = 8
B_FULL = 512
BPC = B_FULL // N_CORES          # 64 sequences per core
T_CORE = BPC * S                 # 4928 tokens per core
G_SEQ = 4                        # sequences per superblock
SB = G_SEQ * S                   # 308 tokens per superblock
MASK_NEG = -1.0e5                # additive causal mask (exp -> exact 0)


def build_program(T=T_CORE, G=G_SEQ):
    import concourse.bass as bass
    import concourse.bacc as bacc
    import concourse.mybir as mybir
    import concourse.tile as tile
    from concourse.masks import make_identity
    from contextlib import ExitStack

    f32 = mybir.dt.float32
    bf16 = mybir.dt.bfloat16
    AX = mybir.AxisListType
    OP = mybir.AluOpType
    AF = mybir.ActivationFunctionType

    SBLK = G * S
    NSB = T // SBLK
    assert NSB * SBLK == T
    NH2 = H // 2                  # 6 head pairs
    SW = NH2 * S                  # 462 score columns per bank
    # token chunks within a superblock
    chunks = []
    off = 0
    while off < SBLK:
        w = min(128, SBLK - off)
        chunks.append((off, w))
        off += w

    nc = bacc.Bacc("TRN2", target_bir_lowering=False)

    x_d = nc.declare_dram_parameter("x", [T, D], f32, isOutput=False)
    wq_d = nc.declare_dram_parameter("wqT", [D, D], bf16, isOutput=False)
    wk_d = nc.declare_dram_parameter("wkT", [D, D], bf16, isOutput=False)
    wv_d = nc.declare_dram_parameter("wvT", [D, D], bf16, isOutput=False)
    wo_d = nc.declare_dram_parameter("woT", [D, D], bf16, isOutput=False)
    wf1_d = nc.declare_dram_parameter("fc1T", [D, FF], bf16, isOutput=False)
    wf2_d = nc.declare_dram_parameter("fc2T", [FF, D], bf16, isOutput=False)
    qb_d = nc.declare_dram_parameter("qb", [D], f32, isOutput=False)
    kb_d = nc.declare_dram_parameter("kb", [D], f32, isOutput=False)
    vb_d = nc.declare_dram_parameter("vb", [D], f32, isOutput=False)
    ob_d = nc.declare_dram_parameter("ob", [D], f32, isOutput=False)
    f1b_d = nc.declare_dram_parameter("fc1b", [FF], f32, isOutput=False)
    f2b_d = nc.declare_dram_parameter("fc2b", [D], f32, isOutput=False)
    mask_d = nc.declare_dram_parameter("mask6", [S, SW], bf16, isOutput=False)
    out_d = nc.declare_dram_parameter("out", [T, D], f32, isOutput=True)

    with tile.TileContext(nc) as tc, ExitStack() as ctx:
        singles = ctx.enter_context(tc.tile_pool(name="singles", bufs=1))
        xpool = ctx.enter_context(tc.tile_pool(name="xpool", bufs=3))
        x2pool = ctx.enter_context(tc.tile_pool(name="x2pool", bufs=3))
        actpool = ctx.enter_context(tc.tile_pool(name="actpool", bufs=1))
        ffpool = ctx.enter_context(tc.tile_pool(name="ffpool", bufs=24))
        outpool = ctx.enter_context(tc.tile_pool(name="outpool", bufs=1))
        attnpool = ctx.enter_context(tc.tile_pool(name="attnpool", bufs=2))
        statpool = ctx.enter_context(tc.tile_pool(name="statpool", bufs=2))
        pspool = ctx.enter_context(tc.tile_pool(name="pspool", bufs=2, space="PSUM"))

        # ---- constants / weights ----
        wq_sb = singles.tile([128, D // 128, D], bf16)
        wk_sb = singles.tile([128, D // 128, D], bf16)
        wv_sb = singles.tile([128, D // 128, D], bf16)
        wo_sb = singles.tile([128, D // 128, D], bf16)
        wf1_sb = singles.tile([128, D // 128, FF], bf16)
        wf2_sb = singles.tile([128, FF // 128, D], bf16)
        for sb_t, dr in ((wq_sb, wq_d), (wk_sb, wk_d), (wv_sb, wv_d),
                         (wo_sb, wo_d), (wf1_sb, wf1_d), (wf2_sb, wf2_d)):
            nc.sync.dma_start(out=sb_t, in_=dr[:].rearrange("(c p) o -> p c o", p=128))

        qb_sb = singles.tile([128, D // 128], f32)
        kb_sb = singles.tile([128, D // 128], f32)
        f1b_sb = singles.tile([128, FF // 128], f32)
        f2b_sb = singles.tile([128, D // 128], f32)
        for sb_t, dr in ((qb_sb, qb_d), (kb_sb, kb_d), (f1b_sb, f1b_d),
                         (f2b_sb, f2b_d)):
            nc.sync.dma_start(out=sb_t, in_=dr[:].rearrange("(c p) -> p c", p=128))

        # free-axis biases broadcast to all 128 partitions
        ob_bc = singles.tile([128, D], f32)
        vb_bc = singles.tile([128, D], f32)
        for sb_t, dr in ((ob_bc, ob_d), (vb_bc, vb_d)):
            src = bass.AP(tensor=dr[:].tensor, offset=dr[:].offset,
                          ap=[[0, 128]] + list(dr[:].ap))
            nc.sync.dma_start(out=sb_t, in_=src)

        mask6_sb = singles.tile([S, SW], bf16)
        nc.sync.dma_start(out=mask6_sb, in_=mask_d[:])

        ident = singles.tile([128, 128], bf16)
        make_identity(nc, ident)

        ones77 = singles.tile([S, 128], bf16)
        nc.vector.memset(ones77, 1.0)

        eps_sb = singles.tile([128, 1], f32)
        nc.vector.memset(eps_sb, EPS)

        NCH = D // 128    # 6
        NFF = FF // 128   # 24

        def ln_normalize(src_tile, w, tag, bufs=2):
            """token-major [w, 768] fp32 -> normalized bf16 htok tile."""
            stats = statpool.tile([128, 3, 6], f32, tag=f"stats{tag}", name=f"stats{tag}")
            mv = statpool.tile([128, 3], f32, tag=f"mv{tag}", name=f"mv{tag}")
            xg = src_tile[:w].rearrange("p (s f) -> p s f", f=256)
            for i in range(3):
                nc.vector.bn_stats(out=stats[:w, i, :], in_=xg[:, i, :])
            nc.vector.bn_aggr(out=mv[:w, 0:2], in_=stats[:w])
            mean = mv[:w, 0:1]
            var = mv[:w, 1:2]
            lnv = mv[:w, 2:3]
            # rstd = exp(-0.5*ln(var+eps)) - stays in the natural_log_exp
            # activation-table set shared with attention's EXP.
            nc.scalar.activation(out=lnv, in_=var, func=AF.Ln,
                                 bias=eps_sb[:w], scale=1.0)
            nc.scalar.activation(out=var, in_=lnv, func=AF.Exp,
                                 bias=0.0, scale=-0.5)
            rstd = var
            htok = statpool.tile([128, D], bf16, tag=f"htok{tag}", name=f"htok{tag}",
                                 bufs=bufs)
            nc.vector.tensor_scalar(out=htok[:w], in0=src_tile[:w],
                                    scalar1=mean, scalar2=rstd,
                                    op0=OP.subtract, op1=OP.mult)
            return htok

        def ln_transpose(htok, coff, w, hT, tag):
            for c in range(NCH):
                ps = pspool.tile([128, 128], bf16, tag="tr", name=f"trp{tag}")
                nc.tensor.transpose(ps[:, :w], htok[:w, c * 128:(c + 1) * 128],
                                    ident[:w, :w])
                nc.any.tensor_copy(out=hT[c][:, coff:coff + w], in_=ps[:, :w])

        def stage_A(isb):
            """load x, LN1 -> hT feature-major bf16; then x_tok += ob (gpsimd)."""
            t0 = isb * SBLK
            hT = [actpool.tile([128, SBLK], bf16, tag=f"hT{c}", name=f"hT{c}", bufs=2)
                  for c in range(NCH)]
            x_tiles = []
            for (coff, w) in chunks:
                x_tok = xpool.tile([128, D], f32, tag="xtok", name="xtok")
                nc.sync.dma_start(out=x_tok[:w], in_=x_d[t0 + coff: t0 + coff + w, :])
                x_tiles.append(x_tok)
                htok = ln_normalize(x_tok, w, "A")
                ln_transpose(htok, coff, w, hT, "A")
                # after LN consumed raw x: fold the o-proj bias into the
                # residual in place on the idle GpSimd engine.
                nc.gpsimd.tensor_tensor(out=x_tok[:w], in0=x_tok[:w],
                                        in1=ob_bc[:w], op=OP.add)
            return hT, x_tiles

        def stage_D_chunk(ci, ctxT, x_tiles, x2_tiles):
            coff, w = chunks[ci]
            x2 = x2pool.tile([128, D], f32, tag="x2tok", name="x2tok")
            for half in range(2):
                ps = pspool.tile([128, 384], f32, tag="big", name="pso")
                for d in range(NCH):
                    nc.tensor.matmul(ps[:w], lhsT=ctxT[d][:, coff:coff + w],
                                     rhs=wo_sb[:, d, half * 384:(half + 1) * 384],
                                     start=(d == 0), stop=(d == NCH - 1))
                sl = slice(half * 384, (half + 1) * 384)
                nc.vector.tensor_tensor(out=x2[:w, sl], in0=ps[:w],
                                        in1=x_tiles[ci][:w, sl], op=OP.add)
            x2_tiles.append(x2)

        cur = stage_A(0)
        for isb in range(NSB):
            t0 = isb * SBLK
            hT, x_tiles = cur

            # ---- stage B: q/k projections (feature-major, bf16) ----
            qT = [actpool.tile([128, SBLK], bf16, tag=f"qT{c}", name=f"qT{c}")
                  for c in range(NCH)]
            kT = [actpool.tile([128, SBLK], bf16, tag=f"kT{c}", name=f"kT{c}")
                  for c in range(NCH)]
            for dst, w_sb, b_sb in ((qT, wq_sb, qb_sb), (kT, wk_sb, kb_sb)):
                for c in range(NCH):
                    ps = pspool.tile([128, SBLK], f32, tag="big", name="psqkv")
                    for d in range(NCH):
                        nc.tensor.matmul(ps, lhsT=w_sb[:, d, c * 128:(c + 1) * 128],
                                         rhs=hT[d], start=(d == 0), stop=(d == NCH - 1))
                    nc.vector.tensor_scalar(out=dst[c], in0=ps,
                                            scalar1=b_sb[:, c:c + 1], scalar2=None,
                                            op0=OP.add)

            # ---- stage C: attention per sequence ----
            ctxT = [actpool.tile([128, SBLK], bf16, tag=f"ctxT{c}", name=f"ctxT{c}",
                                 bufs=2)
                    for c in range(NCH)]
            h2T = [actpool.tile([128, SBLK], bf16, tag=f"h2T{c}", name=f"h2T{c}")
                   for c in range(NCH)]
            x2_tiles = []
            h2toks = []
            next_chunk = 0
            for s in range(G):
                so = s * S
                # v for this sequence, token-major directly (swapped operands)
                vtok = attnpool.tile([S, H, HD], bf16, tag="vtok", name="vtok")
                for half in range(2):
                    psv = pspool.tile([S, 384], f32, tag="big", name="psvtok")
                    for d in range(NCH):
                        nc.tensor.matmul(psv,
                                         lhsT=hT[d][:, so:so + S],
                                         rhs=wv_sb[:, d, half * 384:(half + 1) * 384],
                                         start=(d == 0), stop=(d == NCH - 1))
                    nc.vector.tensor_tensor(
                        out=vtok[:, half * 6:(half + 1) * 6, :], in0=psv,
                        in1=vb_bc[:S, half * 384:(half + 1) * 384], op=OP.add)
                # scoresT[k, q] per head; even heads -> bank A, odd -> bank B.
                # Row-group packing: even heads live on partitions 0:64 of
                # their qT/kT chunk, odd heads on 64:128 -> pairs overlap.
                psc = [pspool.tile([S, SW], f32, tag="sc", name=f"psc{a}")
                       for a in range(2)]
                for j in range(NH2):
                    for a in range(2):
                        h = 2 * j + a
                        c, po = h // 2, 64 * (h % 2)
                        nc.tensor.matmul(psc[a][:, j * S:(j + 1) * S],
                                         lhsT=kT[c][po:po + 64, so:so + S],
                                         rhs=qT[c][po:po + 64, so:so + S],
                                         start=(j == 0), stop=False,
                                         skip_group_check=True)
                # additive causal mask via identity-matmul accumulation
                pT = attnpool.tile([S, 2, SW], bf16, tag="pT", name="pT")
                for a in range(2):
                    nc.tensor.matmul(psc[a], lhsT=ident[:S, :S], rhs=mask6_sb,
                                     start=False, stop=True, skip_group_check=True)
                    nc.scalar.activation(out=pT[:, a, :], in_=psc[a], func=AF.Exp)
                # denominators broadcast across partitions: ones.T @ pT, the
                # two banks column-packed into one PSUM tile (concurrent).
                dben = pspool.tile([128, SW], f32, tag="sc", name="dben")
                rp = attnpool.tile([128, SW], bf16, tag="rp", name="rp")
                lnd = attnpool.tile([128, SW], f32, tag="lnd", name="lnd")
                for a in range(2):
                    nc.tensor.matmul(dben[64 * a:64 * a + 64, :],
                                     lhsT=ones77[:, 64 * a:64 * a + 64],
                                     rhs=pT[:, a, :],
                                     start=True, stop=True,
                                     skip_group_check=True)
                # reciprocal as exp(-ln(x)) on the scalar engine: same table
                # set as the attention EXP, and off the busy vector engine
                # (nc.vector.reciprocal is ~6.5ns/elem - 3us per call here).
                for a in range(2):
                    sl = slice(64 * a, 64 * a + 64)
                    nc.scalar.activation(out=lnd[sl], in_=dben[sl], func=AF.Ln)
                    nc.scalar.activation(out=rp[sl], in_=lnd[sl], func=AF.Exp,
                                         scale=-1.0)
                # ctx per head pair, column-packed into [128, 77] PSUM already
                # in ctxT layout; normalization fused into the evacuation.
                for j in range(NH2):
                    ctxp = pspool.tile([128, S], f32, tag="ctxp", name="ctxp")
                    for a in range(2):
                        h = 2 * j + a
                        nc.tensor.matmul(ctxp[64 * a:64 * a + 64, :],
                                         lhsT=vtok[:, h, :],
                                         rhs=pT[:, a, j * S:(j + 1) * S],
                                         start=True, stop=True,
                                         skip_group_check=True)
                    nc.vector.tensor_tensor(out=ctxT[j][:, so:so + S], in0=ctxp,
                                            in1=rp[:, j * S:(j + 1) * S],
                                            op=OP.mult)
                # emit O-proj + residual + LN2 for chunks fully covered
                done_tokens = (s + 1) * S
                while (next_chunk < len(chunks)
                       and chunks[next_chunk][0] + chunks[next_chunk][1]
                       <= done_tokens):
                    ci = next_chunk
                    stage_D_chunk(ci, ctxT, x_tiles, x2_tiles)
                    coff, w = chunks[ci]
                    h2toks.append(ln_normalize(x2_tiles[ci], w, "E", bufs=3))
                    next_chunk += 1

            # E transposes (emitted after C so the in-order PE isn't blocked
            # mid-attention waiting on the LN chains)
            for ci, (coff, w) in enumerate(chunks):
                ln_transpose(h2toks[ci], coff, w, h2T, "E")

            # ---- stage F: MLP ----
            ff1 = []
            for f in range(NFF):
                ps = pspool.tile([128, SBLK], f32, tag="big", name="psff")
                for d in range(NCH):
                    nc.tensor.matmul(ps, lhsT=wf1_sb[:, d, f * 128:(f + 1) * 128],
                                     rhs=h2T[d], start=(d == 0), stop=(d == NCH - 1))
                f1 = ffpool.tile([128, SBLK], bf16, tag="ff1", name="ff1")
                # f1 = silu(1.702*ps + 1.702*b) = 1.702*quickgelu(ps+b);
                # the 1/1.702 is folded into fc2T host-side.
                nc.scalar.activation(out=f1, in_=ps, func=AF.Silu,
                                     bias=f1b_sb[:, f:f + 1], scale=1.702)
                ff1.append(f1)
            # FC2 feature-major (weights stationary, tokens streaming), then
            # PE-transpose each dout chunk back to token-major with the final
            # residual fused into the PSUM evacuation add.
            o_toks = [outpool.tile([128, D], f32, tag=f"otok{ci}", name=f"otok{ci}")
                      for ci in range(len(chunks))]
            for c in range(NCH):
                ps = pspool.tile([128, SBLK], f32, tag="big", name="psf2")
                for f in range(NFF):
                    nc.tensor.matmul(ps, lhsT=wf2_sb[:, f, c * 128:(c + 1) * 128],
                                     rhs=ff1[f], start=(f == 0), stop=(f == NFF - 1))
                x3c = statpool.tile([128, SBLK], bf16, tag="x3", name="x3")
                nc.vector.tensor_scalar(out=x3c, in0=ps,
                                        scalar1=f2b_sb[:, c:c + 1], scalar2=None,
                                        op0=OP.add)
                for ci, (coff, w) in enumerate(chunks):
                    tr = pspool.tile([128, 128], bf16, tag="tr", name="trf")
                    nc.tensor.transpose(tr[:w, :], x3c[:, coff:coff + w], ident)
                    nc.vector.tensor_tensor(
                        out=o_toks[ci][:w, c * 128:(c + 1) * 128],
                        in0=tr[:w, :],
                        in1=x2_tiles[ci][:w, c * 128:(c + 1) * 128], op=OP.add)
            for ci, (coff, w) in enumerate(chunks):
                nc.sync.dma_start(out=out_d[t0 + coff: t0 + coff + w, :],
                                  in_=o_toks[ci][:w])

            # ---- prefetch next superblock's stage A (overlaps F on DVE) ----
            if isb + 1 < NSB:
                cur = stage_A(isb + 1)

    # Restrict the activation-table-set chooser to the two sets that cover
    # everything this kernel uses (ln+exp share one set; silu the other).
    # Entries keep their original indices (act_func_set_id is positional);
    # unwanted sets are just emptied so the chooser can never pick them.
    from concourse.hw_specs import get_activation_tables
    import bass_rust as _bass_rust
    _tables = list(get_activation_tables(nc.m.arch).items())
    _keep = {"natural_log_exp_and_others", "silu_and_others"}
    _tables = [(n, (f if n in _keep else set())) for (n, f) in _tables]

    def _patched_insert_act_table_loads():
        _bass_rust.insert_act_table_loads(nc, _tables)

    nc.insert_act_table_loads = _patched_insert_act_table_loads
    nc.compile()
    return nc


def prep_shared(inputs):
    """Fold LN affine params / scale constants into weights -> shared in_map entries."""
    bf = ml_dtypes.bfloat16
    f32 = np.float32
    g = {k: np.asarray(v, dtype=np.float32) for k, v in inputs.items() if k != "x"}

    wqT = (g["ln1_w"][:, None] * g["qw"].T * 0.125).astype(bf)
    wkT = (g["ln1_w"][:, None] * g["kw"].T).astype(bf)
    wvT = (g["ln1_w"][:, None] * g["vw"].T).astype(bf)
    woT = np.ascontiguousarray(g["ow"].T).astype(bf)
    fc1T = (g["ln2_w"][:, None] * g["fc1_w"].T).astype(bf)
    fc2T = (g["fc2_w"].T / 1.702).astype(bf)

    qb = ((g["ln1_b"] @ g["qw"].T + g["qb"]) * 0.125).astype(f32)
    kb = (g["ln1_b"] @ g["kw"].T + g["kb"]).astype(f32)
    vb = (g["ln1_b"] @ g["vw"].T + g["vb"]).astype(f32)
    ob = g["ob"].astype(f32)
    fc1b = ((g["ln2_b"] @ g["fc1_w"].T + g["fc1_b"]) * 1.702).astype(f32)
    fc2b = g["fc2_b"].astype(f32)

    # additive causal mask in scoresT[k, q] orientation (k > q masked),
    # tiled 6x along q for the per-bank [77, 462] accumulation matmul.
    m1 = np.where(np.arange(S)[:, None] > np.arange(S)[None, :], MASK_NEG, 0.0)
    mask6 = np.tile(m1.astype(np.float32), (1, H // 2)).astype(bf)

    return dict(wqT=wqT, wkT=wkT, wvT=wvT, woT=woT, fc1T=fc1T, fc2T=fc2T,
                qb=qb, kb=kb, vb=vb, ob=ob, fc1b=fc1b, fc2b=fc2b, mask6=mask6)


def prep_host_inputs(inputs):
    shared = prep_shared(inputs)
    x = np.asarray(inputs["x"], dtype=np.float32)
    in_maps = []
    for c in range(N_CORES):
        xc = np.ascontiguousarray(
            x[c * BPC:(c + 1) * BPC].reshape(T_CORE, D).astype(np.float32))
        in_maps.append(dict(shared, x=xc))
    return in_maps


_CACHED_NC = None


def _get_nc():
    global _CACHED_NC
    if _CACHED_NC is None:
        _CACHED_NC = build_program()
    return _CACHED_NC


def run(inputs, trace=False):
    from concourse.bass_utils import run_bass_kernel_spmd
    nc = _get_nc()
    in_maps = prep_host_inputs(inputs)
    res = run_bass_kernel_spmd(nc, in_maps, list(range(N_CORES)), trace=trace)
    outs = [np.asarray(res.results[c]["out"], dtype=np.float32).reshape(BPC, S, D)
            for c in range(N_CORES)]
    full = np.concatenate(outs, axis=0)
    return full, res


def kernel(**inputs):
    full, _ = run(inputs, trace=False)
    return full


# revision 16
# speedup vs baseline: 1.4879x; 1.2229x over previous
"""CLIP encoder layer on 8 trn2 NeuronCores, pure data parallel over batch.

Layout strategy (per core, batch shard of 64 sequences = 4928 tokens):
  - x arrives token-major [T, 768] fp32.
  - LayerNorm runs token-major (tokens on partitions, bn_stats/bn_aggr),
    LN scale/bias folded into the downstream projection weights host-side.
    rstd computed as exp(-0.5*ln(var+eps)) so LN shares the scalar-engine
    natural_log_exp table set with attention's EXP (no sqrt-set thrash).
  - Normalized activations are PE-transposed (bf16) to feature-major
    [768, N] for the projections (weights stationary, activations moving).
  - Attention per sequence (S=77): scoresT[k,q] = kT.T @ qT per head,
    written directly in k-major orientation so no post-softmax transpose is
    needed; 6 even heads share one PSUM bank, 6 odd heads another (row-group
    packed pairs run concurrently in the PE).  The causal mask is ADDED via
    an identity-matmul accumulation (-1e5 above diagonal) before EXP, so the
    only cross-engine dependency between scores and ctx is a single EXP per
    bank.  Softmax denominators come out of a ones-matrix matmul
    (ones77.T @ pT -> every partition holds the per-query denominator);
    normalization is fused into the ctx PSUM->SBUF evacuation multiply.
  - ctx matmuls are column-packed pairs writing [128, 77] PSUM tiles that
    are already in the ctxT feature-major layout the O-projection wants.
  - O-projection runs with swapped operands (activations stationary) so its
    output comes out token-major for the residual add (x+ob precomputed on
    the otherwise-idle GpSimd engine) and the second LayerNorm.
  - FC1 feature-major (weights stationary).  FC2 also feature-major
    (44K vs 55K PE cycles), then PE-transposed back to token-major with the
    final residual add fused into the PSUM evacuation.
  - All matmuls in bf16 (fp32 PSUM accumulation); fp32 elsewhere.
    QuickGELU via ACT Silu: x*sigmoid(1.702x) = silu(1.702x)/1.702 with the
    1/1.702 folded into fc2 weights and the 1.702 into the ACT input scale.
"""

import os
import numpy as np
import ml_dtypes

D = 768
H = 12
HD = 64
S = 77
FF = 3072
EPS = 1e-5
N_CORES = 8
B_FULL = 512
BPC = B_FULL // N_CORES          # 64 sequences per core
T_CORE = BPC * S                 # 4928 tokens per core
G_SEQ = 4                        # sequences per superblock
SB = G_SEQ * S                   # 308 tokens per superblock
MASK_NEG = -1.0e5                # additive causal mask (exp -> exact 0)


def build_program(T=T_CORE, G=G_SEQ):
    import concourse.bass as bass
    import concourse.bacc as bacc
    import concourse.mybir as mybir
    import concourse.tile as tile
    from concourse.masks import make_identity
    from contextlib import ExitStack

    f32 = mybir.dt.float32
    bf16 = mybir.dt.bfloat16
    AX = mybir.AxisListType
    OP = mybir.AluOpType
    AF = mybir.ActivationFunctionType

    SBLK = G * S
    NSB = T // SBLK
    assert NSB * SBLK == T
    NH2 = H // 2                  # 6 head pairs
    SW = NH2 * S                  # 462 score columns per bank
    # token chunks within a superblock
    chunks = []
    off = 0
    while off < SBLK:
        w = min(128, SBLK - off)
        chunks.append((off, w))
        off += w

    nc = bacc.Bacc("TRN2", target_bir_lowering=False)

    x_d = nc.declare_dram_parameter("x", [T, D], f32, isOutput=False)
    wq_d = nc.declare_dram_parameter("wqT", [D, D], bf16, isOutput=False)
    wk_d = nc.declare_dram_parameter("wkT", [D, D], bf16, isOutput=False)
    wv_d = nc.declare_dram_parameter("wvT", [D, D], bf16, isOutput=False)
    wo_d = nc.declare_dram_parameter("woT", [D, D], bf16, isOutput=False)
    fp8 = mybir.dt.float8e4
    wf1_d = nc.declare_dram_parameter("fc1T", [D, FF], fp8, isOutput=False)
    wf2_d = nc.declare_dram_parameter("fc2T", [FF, D], fp8, isOutput=False)
    qb_d = nc.declare_dram_parameter("qb", [D], f32, isOutput=False)
    kb_d = nc.declare_dram_parameter("kb", [D], f32, isOutput=False)
    vb_d = nc.declare_dram_parameter("vb", [D], f32, isOutput=False)
    ob_d = nc.declare_dram_parameter("ob", [D], f32, isOutput=False)
    f1b_d = nc.declare_dram_parameter("fc1b", [FF], f32, isOutput=False)
    f2b_d = nc.declare_dram_parameter("fc2b", [D], f32, isOutput=False)
    mask_d = nc.declare_dram_parameter("mask6", [S, SW], bf16, isOutput=False)
    out_d = nc.declare_dram_parameter("out", [T, D], f32, isOutput=True)

    with tile.TileContext(nc) as tc, ExitStack() as ctx:
        singles = ctx.enter_context(tc.tile_pool(name="singles", bufs=1))
        xpool = ctx.enter_context(tc.tile_pool(name="xpool", bufs=3))
        x2pool = ctx.enter_context(tc.tile_pool(name="x2pool", bufs=3))
        actpool = ctx.enter_context(tc.tile_pool(name="actpool", bufs=1))
        outpool = ctx.enter_context(tc.tile_pool(name="outpool", bufs=1))
        attnpool = ctx.enter_context(tc.tile_pool(name="attnpool", bufs=2))
        statpool = ctx.enter_context(tc.tile_pool(name="statpool", bufs=2))
        pspool = ctx.enter_context(tc.tile_pool(name="pspool", bufs=2, space="PSUM"))

        # ---- constants / weights ----
        wq_sb = singles.tile([128, D // 128, D], bf16)
        wk_sb = singles.tile([128, D // 128, D], bf16)
        wv_sb = singles.tile([128, D // 128, D], bf16)
        wo_sb = singles.tile([128, D // 128, D], bf16)
        wf1_sb = singles.tile([128, D // 128, FF], fp8)
        wf2_sb = singles.tile([128, FF // 128, D], fp8)
        for sb_t, dr in ((wq_sb, wq_d), (wk_sb, wk_d), (wv_sb, wv_d),
                         (wo_sb, wo_d), (wf1_sb, wf1_d), (wf2_sb, wf2_d)):
            nc.sync.dma_start(out=sb_t, in_=dr[:].rearrange("(c p) o -> p c o", p=128))

        qb_sb = singles.tile([128, D // 128], f32)
        kb_sb = singles.tile([128, D // 128], f32)
        f1b_sb = singles.tile([128, FF // 128], f32)
        f2b_sb = singles.tile([128, D // 128], f32)
        for sb_t, dr in ((qb_sb, qb_d), (kb_sb, kb_d), (f1b_sb, f1b_d),
                         (f2b_sb, f2b_d)):
            nc.sync.dma_start(out=sb_t, in_=dr[:].rearrange("(c p) -> p c", p=128))

        # free-axis biases broadcast to all 128 partitions
        ob_bc = singles.tile([128, D], f32)
        vb_bc = singles.tile([128, D], f32)
        for sb_t, dr in ((ob_bc, ob_d), (vb_bc, vb_d)):
            src = bass.AP(tensor=dr[:].tensor, offset=dr[:].offset,
                          ap=[[0, 128]] + list(dr[:].ap))
            nc.sync.dma_start(out=sb_t, in_=src)

        mask6_sb = singles.tile([S, SW], bf16)
        nc.sync.dma_start(out=mask6_sb, in_=mask_d[:])

        ident = singles.tile([128, 128], bf16)
        make_identity(nc, ident)

        ones77 = singles.tile([S, 128], bf16)
        nc.vector.memset(ones77, 1.0)

        eps_sb = singles.tile([128, 1], f32)
        nc.vector.memset(eps_sb, EPS)

        NCH = D // 128    # 6
        NFF = FF // 128   # 24

        def ln_normalize(src_tile, w, tag, bufs=2):
            """token-major [w, 768] fp32 -> normalized bf16 htok tile."""
            stats = statpool.tile([128, 3, 6], f32, tag=f"stats{tag}", name=f"stats{tag}")
            mv = statpool.tile([128, 3], f32, tag=f"mv{tag}", name=f"mv{tag}")
            xg = src_tile[:w].rearrange("p (s f) -> p s f", f=256)
            for i in range(3):
                nc.vector.bn_stats(out=stats[:w, i, :], in_=xg[:, i, :])
            nc.vector.bn_aggr(out=mv[:w, 0:2], in_=stats[:w])
            mean = mv[:w, 0:1]
            var = mv[:w, 1:2]
            lnv = mv[:w, 2:3]
            # rstd = exp(-0.5*ln(var+eps)) - stays in the natural_log_exp
            # activation-table set shared with attention's EXP.
            nc.scalar.activation(out=lnv, in_=var, func=AF.Ln,
                                 bias=eps_sb[:w], scale=1.0)
            nc.scalar.activation(out=var, in_=lnv, func=AF.Exp,
                                 bias=0.0, scale=-0.5)
            rstd = var
            htok = statpool.tile([128, D], bf16, tag=f"htok{tag}", name=f"htok{tag}",
                                 bufs=bufs)
            nc.vector.tensor_scalar(out=htok[:w], in0=src_tile[:w],
                                    scalar1=mean, scalar2=rstd,
                                    op0=OP.subtract, op1=OP.mult)
            return htok

        def ln_transpose(htok, coff, w, dst, tag):
            for c in range(NCH):
                ps = pspool.tile([128, 128], bf16, tag="tr", name=f"trp{tag}")
                nc.tensor.transpose(ps[:, :w], htok[:w, c * 128:(c + 1) * 128],
                                    ident[:w, :w])
                nc.vector.tensor_copy(out=dst(c, coff, w), in_=ps[:, :w])

        def stage_A(isb):
            """load x, LN1 -> hT feature-major bf16; then x_tok += ob (gpsimd)."""
            t0 = isb * SBLK
            hT = [actpool.tile([128, SBLK], bf16, tag=f"hT{c}", name=f"hT{c}", bufs=2)
                  for c in range(NCH)]
            x_tiles = []
            for (coff, w) in chunks:
                x_tok = xpool.tile([128, D], f32, tag="xtok", name="xtok")
                nc.sync.dma_start(out=x_tok[:w], in_=x_d[t0 + coff: t0 + coff + w, :])
                x_tiles.append(x_tok)
                htok = ln_normalize(x_tok, w, "A")
                ln_transpose(htok, coff, w,
                             lambda c, o, ww: hT[c][:, o:o + ww], "A")
                # after LN consumed raw x: fold the o-proj bias into the
                # residual in place on the idle GpSimd engine.
                nc.gpsimd.tensor_tensor(out=x_tok[:w], in0=x_tok[:w],
                                        in1=ob_bc[:w], op=OP.add)
            return hT, x_tiles

        def stage_D_chunk(ci, ctxT, x_tiles, x2_tiles):
            coff, w = chunks[ci]
            x2 = x2pool.tile([128, D], f32, tag="x2tok", name="x2tok")
            for half in range(2):
                ps = pspool.tile([128, 384], f32, tag="big", name="pso")
                for d in range(NCH):
                    nc.tensor.matmul(ps[:w], lhsT=ctxT[d][:, coff:coff + w],
                                     rhs=wo_sb[:, d, half * 384:(half + 1) * 384],
                                     start=(d == 0), stop=(d == NCH - 1))
                sl = slice(half * 384, (half + 1) * 384)
                nc.vector.tensor_tensor(out=x2[:w, sl], in0=ps[:w],
                                        in1=x_tiles[ci][:w, sl], op=OP.add)
            x2_tiles.append(x2)

        cur = stage_A(0)
        for isb in range(NSB):
            t0 = isb * SBLK
            hT, x_tiles = cur

            # ---- stage B: q/k projections (feature-major, bf16) ----
            qT = [actpool.tile([128, SBLK], bf16, tag=f"qT{c}", name=f"qT{c}")
                  for c in range(NCH)]
            kT = [actpool.tile([128, SBLK], bf16, tag=f"kT{c}", name=f"kT{c}")
                  for c in range(NCH)]
            for dst, w_sb, b_sb in ((qT, wq_sb, qb_sb), (kT, wk_sb, kb_sb)):
                for c in range(NCH):
                    ps = pspool.tile([128, SBLK], f32, tag="big", name="psqkv")
                    for d in range(NCH):
                        nc.tensor.matmul(ps, lhsT=w_sb[:, d, c * 128:(c + 1) * 128],
                                         rhs=hT[d], start=(d == 0), stop=(d == NCH - 1))
                    nc.vector.tensor_scalar(out=dst[c], in0=ps,
                                            scalar1=b_sb[:, c:c + 1], scalar2=None,
                                            op0=OP.add)

            # ---- stage C: attention per sequence ----
            ctxT = [actpool.tile([128, SBLK], bf16, tag=f"ctxT{c}", name=f"ctxT{c}",
                                 bufs=2)
                    for c in range(NCH)]
            # fp8 feature-major LN2 activations for the DoubleRow FC1
            # (padded to 320 so the k-pair free step is 16B-aligned)
            h2T8 = actpool.tile([128, NCH, 320], fp8, tag="h2T8", name="h2T8")
            x2_tiles = []
            h2toks = []
            next_chunk = 0
            for s in range(G):
                so = s * S
                # v for this sequence, token-major directly (swapped operands)
                vtok = attnpool.tile([S, H, HD], bf16, tag="vtok", name="vtok")
                for half in range(2):
                    psv = pspool.tile([S, 384], f32, tag="big", name="psvtok")
                    for d in range(NCH):
                        nc.tensor.matmul(psv,
                                         lhsT=hT[d][:, so:so + S],
                                         rhs=wv_sb[:, d, half * 384:(half + 1) * 384],
                                         start=(d == 0), stop=(d == NCH - 1))
                    nc.vector.tensor_tensor(
                        out=vtok[:, half * 6:(half + 1) * 6, :], in0=psv,
                        in1=vb_bc[:S, half * 384:(half + 1) * 384], op=OP.add)
                # scoresT[k, q] per head; even heads -> bank A, odd -> bank B.
                # Row-group packing: even heads live on partitions 0:64 of
                # their qT/kT chunk, odd heads on 64:128 -> pairs overlap.
                psc = [pspool.tile([S, SW], f32, tag="sc", name=f"psc{a}")
                       for a in range(2)]
                for j in range(NH2):
                    for a in range(2):
                        h = 2 * j + a
                        c, po = h // 2, 64 * (h % 2)
                        nc.tensor.matmul(psc[a][:, j * S:(j + 1) * S],
                                         lhsT=kT[c][po:po + 64, so:so + S],
                                         rhs=qT[c][po:po + 64, so:so + S],
                                         start=(j == 0), stop=False,
                                         skip_group_check=True)
                # additive causal mask via identity-matmul accumulation
                pT = attnpool.tile([S, 2, SW], bf16, tag="pT", name="pT")
                for a in range(2):
                    nc.tensor.matmul(psc[a], lhsT=ident[:S, :S], rhs=mask6_sb,
                                     start=False, stop=True, skip_group_check=True)
                    nc.scalar.activation(out=pT[:, a, :], in_=psc[a], func=AF.Exp)
                # denominators broadcast across partitions: ones.T @ pT, the
                # two banks column-packed into one PSUM tile (concurrent).
                dben = pspool.tile([128, SW], f32, tag="sc", name="dben")
                rp = attnpool.tile([128, SW], bf16, tag="rp", name="rp")
                lnd = attnpool.tile([128, SW], f32, tag="lnd", name="lnd")
                for a in range(2):
                    nc.tensor.matmul(dben[64 * a:64 * a + 64, :],
                                     lhsT=ones77[:, 64 * a:64 * a + 64],
                                     rhs=pT[:, a, :],
                                     start=True, stop=True,
                                     skip_group_check=True)
                # reciprocal as exp(-ln(x)) on the scalar engine: same table
                # set as the attention EXP, and off the busy vector engine
                # (nc.vector.reciprocal is ~6.5ns/elem - 3us per call here).
                for a in range(2):
                    sl = slice(64 * a, 64 * a + 64)
                    nc.scalar.activation(out=lnd[sl], in_=dben[sl], func=AF.Ln)
                    nc.scalar.activation(out=rp[sl], in_=lnd[sl], func=AF.Exp,
                                         scale=-1.0)
                # ctx per head pair, column-packed into [128, 77] PSUM already
                # in ctxT layout; normalization fused into the evacuation.
                for j in range(NH2):
                    ctxp = pspool.tile([128, S], f32, tag="ctxp", name="ctxp")
                    for a in range(2):
                        h = 2 * j + a
                        nc.tensor.matmul(ctxp[64 * a:64 * a + 64, :],
                                         lhsT=vtok[:, h, :],
                                         rhs=pT[:, a, j * S:(j + 1) * S],
                                         start=True, stop=True,
                                         skip_group_check=True)
                    nc.vector.tensor_tensor(out=ctxT[j][:, so:so + S], in0=ctxp,
                                            in1=rp[:, j * S:(j + 1) * S],
                                            op=OP.mult)
                # emit O-proj + residual + LN2 for chunks fully covered
                done_tokens = (s + 1) * S
                while (next_chunk < len(chunks)
                       and chunks[next_chunk][0] + chunks[next_chunk][1]
                       <= done_tokens):
                    ci = next_chunk
                    stage_D_chunk(ci, ctxT, x_tiles, x2_tiles)
                    coff, w = chunks[ci]
                    h2toks.append(ln_normalize(x2_tiles[ci], w, "E", bufs=3))
                    next_chunk += 1

            # ---- prefetch next superblock's stage A here: its scalar ln/exp
            # ops stay adjacent to attention's (same table set), its
            # transposes keep the PE warm through the E/F transition ----
            if isb + 1 < NSB:
                cur = stage_A(isb + 1)

            # E transposes (emitted after C so the in-order PE isn't blocked
            # mid-attention waiting on the LN chains)
            for ci, (coff, w) in enumerate(chunks):
                ln_transpose(h2toks[ci], coff, w,
                             lambda c, o, ww: h2T8[:, c, o:o + ww], "E")

            # ---- stage F: MLP (fp8 DoubleRow matmuls, weights 16x-scaled
            # host-side; two 128-contractions fused per instruction) ----
            DR = mybir.MatmulPerfMode.DoubleRow
            ff1_8 = actpool.tile([128, NFF, 320], fp8, tag="ff18", name="ff18")
            for f in range(NFF):
                ps = pspool.tile([128, SBLK], f32, tag="big", name="psff")
                for dp in range(NCH // 2):
                    nc.tensor.matmul(ps,
                                     lhsT=wf1_sb[:, 2 * dp:2 * dp + 2,
                                                 f * 128:(f + 1) * 128],
                                     rhs=h2T8[:, 2 * dp:2 * dp + 2, :SBLK],
                                     perf_mode=DR,
                                     start=(dp == 0), stop=(dp == NCH // 2 - 1))
                # f1 = silu(1.702*(ps/16) + 1.702*b) = 1.702*quickgelu(ps+b);
                # the 1/1.702 is folded into fc2T host-side.
                nc.scalar.activation(out=ff1_8[:, f, :SBLK], in_=ps, func=AF.Silu,
                                     bias=f1b_sb[:, f:f + 1], scale=1.702 / 16)
            # FC2 feature-major (weights stationary, tokens streaming), then
            # PE-transpose each dout chunk back to token-major with the final
            # residual fused into the PSUM evacuation add.
            o_toks = [outpool.tile([128, D], f32, tag=f"otok{ci}", name=f"otok{ci}")
                      for ci in range(len(chunks))]
            for c in range(NCH):
                ps = pspool.tile([128, SBLK], f32, tag="big", name="psf2")
                for fp in range(NFF // 2):
                    nc.tensor.matmul(ps,
                                     lhsT=wf2_sb[:, 2 * fp:2 * fp + 2,
                                                 c * 128:(c + 1) * 128],
                                     rhs=ff1_8[:, 2 * fp:2 * fp + 2, :SBLK],
                                     perf_mode=DR,
                                     start=(fp == 0), stop=(fp == NFF // 2 - 1))
                x3c = statpool.tile([128, SBLK], bf16, tag="x3", name="x3")
                nc.vector.tensor_scalar(out=x3c, in0=ps,
                                        scalar1=1.0 / 16, scalar2=f2b_sb[:, c:c + 1],
                                        op0=OP.mult, op1=OP.add)
                for ci, (coff, w) in enumerate(chunks):
                    tr = pspool.tile([128, 128], bf16, tag="tr", name="trf")
                    nc.tensor.transpose(tr[:w, :], x3c[:, coff:coff + w], ident)
                    nc.vector.tensor_tensor(
                        out=o_toks[ci][:w, c * 128:(c + 1) * 128],
                        in0=tr[:w, :],
                        in1=x2_tiles[ci][:w, c * 128:(c + 1) * 128], op=OP.add)
            for ci, (coff, w) in enumerate(chunks):
                nc.sync.dma_start(out=out_d[t0 + coff: t0 + coff + w, :],
                                  in_=o_toks[ci][:w])

    # Restrict the activation-table-set chooser to the two sets that cover
    # everything this kernel uses (ln+exp share one set; silu the other).
    # Entries keep their original indices (act_func_set_id is positional);
    # unwanted sets are just emptied so the chooser can never pick them.
    from concourse.hw_specs import get_activation_tables
    import bass_rust as _bass_rust
    _tables = list(get_activation_tables(nc.m.arch).items())
    _keep = {"natural_log_exp_and_others", "silu_and_others"}
    _tables = [(n, (f if n in _keep else set())) for (n, f) in _tables]

    def _patched_insert_act_table_loads():
        _bass_rust.insert_act_table_loads(nc, _tables)

    nc.insert_act_table_loads = _patched_insert_act_table_loads
    nc.compile()
    return nc


def prep_shared(inputs):
    """Fold LN affine params / scale constants into weights -> shared in_map entries."""
    bf = ml_dtypes.bfloat16
    f32 = np.float32
    g = {k: np.asarray(v, dtype=np.float32) for k, v in inputs.items() if k != "x"}

    wqT = (g["ln1_w"][:, None] * g["qw"].T * 0.125).astype(bf)
    wkT = (g["ln1_w"][:, None] * g["kw"].T).astype(bf)
    wvT = (g["ln1_w"][:, None] * g["vw"].T).astype(bf)
    woT = np.ascontiguousarray(g["ow"].T).astype(bf)
    # fc1/fc2 weights in fp8 e4m3, scaled 16x so the smallest weights stay
    # out of the subnormal floor; the 1/16 is folded into the PSUM
    # evacuations (SILU input scale / fc2 bias tensor_scalar).
    e4 = ml_dtypes.float8_e4m3
    fc1T = (g["ln2_w"][:, None] * g["fc1_w"].T * 16.0).astype(e4)
    fc2T = (g["fc2_w"].T / 1.702 * 16.0).astype(e4)

    qb = ((g["ln1_b"] @ g["qw"].T + g["qb"]) * 0.125).astype(f32)
    kb = (g["ln1_b"] @ g["kw"].T + g["kb"]).astype(f32)
    vb = (g["ln1_b"] @ g["vw"].T + g["vb"]).astype(f32)
    ob = g["ob"].astype(f32)
    fc1b = ((g["ln2_b"] @ g["fc1_w"].T + g["fc1_b"]) * 1.702).astype(f32)
    fc2b = g["fc2_b"].astype(f32)

    # additive causal mask in scoresT[k, q] orientation (k > q masked),
    # tiled 6x along q for the per-bank [77, 462] accumulation matmul.
    m1 = np.where(np.arange(S)[:, None] > np.arange(S)[None, :], MASK_NEG, 0.0)
    mask6 = np.tile(m1.astype(np.float32), (1, H // 2)).astype(bf)

    return dict(wqT=wqT, wkT=wkT, wvT=wvT, woT=woT, fc1T=fc1T, fc2T=fc2T,
                qb=qb, kb=kb, vb=vb, ob=ob, fc1b=fc1b, fc2b=fc2b, mask6=mask6)


def prep_host_inputs(inputs):
    shared = prep_shared(inputs)
    x = np.asarray(inputs["x"], dtype=np.float32)
    in_maps = []
    for c in range(N_CORES):
        xc = np.ascontiguousarray(
            x[c * BPC:(c + 1) * BPC].reshape(T_CORE, D).astype(np.float32))
        in_maps.append(dict(shared, x=xc))
    return in_maps


_CACHED_NC = None


def _get_nc():
    global _CACHED_NC
    if _CACHED_NC is None:
        _CACHED_NC = build_program()
    return _CACHED_NC


def run(inputs, trace=False):
    from concourse.bass_utils import run_bass_kernel_spmd
    nc = _get_nc()
    in_maps = prep_host_inputs(inputs)
    res = run_bass_kernel_spmd(nc, in_maps, list(range(N_CORES)), trace=trace)
    outs = [np.asarray(res.results[c]["out"], dtype=np.float32).reshape(BPC, S, D)
            for c in range(N_CORES)]
    full = np.concatenate(outs, axis=0)
    return full, res


def kernel(**inputs):
    full, _ = run(inputs, trace=False)
    return full


# revision 29
# speedup vs baseline: 1.5266x; 1.0260x over previous
"""CLIP encoder layer on 8 trn2 NeuronCores, pure data parallel over batch.

Layout strategy (per core, batch shard of 64 sequences = 4928 tokens):
  - x arrives token-major [T, 768] fp32.
  - LayerNorm runs token-major (tokens on partitions, bn_stats/bn_aggr),
    LN scale/bias folded into the downstream projection weights host-side.
    rstd computed as exp(-0.5*ln(var+eps)) so LN shares the scalar-engine
    natural_log_exp table set with attention's EXP (no sqrt-set thrash).
  - Normalized activations are PE-transposed (bf16) to feature-major
    [768, N] for the projections (weights stationary, activations moving).
  - Attention per sequence (S=77): scoresT[k,q] = kT.T @ qT per head,
    written directly in k-major orientation so no post-softmax transpose is
    needed; 6 even heads share one PSUM bank, 6 odd heads another (row-group
    packed pairs run concurrently in the PE).  The causal mask is ADDED via
    an identity-matmul accumulation (-1e5 above diagonal) before EXP, so the
    only cross-engine dependency between scores and ctx is a single EXP per
    bank.  Softmax denominators come out of a ones-matrix matmul
    (ones77.T @ pT -> every partition holds the per-query denominator);
    normalization is fused into the ctx PSUM->SBUF evacuation multiply.
  - ctx matmuls are column-packed pairs writing [128, 77] PSUM tiles that
    are already in the ctxT feature-major layout the O-projection wants.
  - O-projection runs with swapped operands (activations stationary) so its
    output comes out token-major for the residual add (x+ob precomputed on
    the otherwise-idle GpSimd engine) and the second LayerNorm.
  - FC1 feature-major (weights stationary).  FC2 also feature-major
    (44K vs 55K PE cycles), then PE-transposed back to token-major with the
    final residual add fused into the PSUM evacuation.
  - All matmuls in bf16 (fp32 PSUM accumulation); fp32 elsewhere.
    QuickGELU via ACT Silu: x*sigmoid(1.702x) = silu(1.702x)/1.702 with the
    1/1.702 folded into fc2 weights and the 1.702 into the ACT input scale.
"""

import os
import numpy as np
import ml_dtypes

D = 768
H = 12
HD = 64
S = 77
FF = 3072
EPS = 1e-5
N_CORES = 8
B_FULL = 512
BPC = B_FULL // N_CORES          # 64 sequences per core
T_CORE = BPC * S                 # 4928 tokens per core
G_SEQ = 4                        # sequences per superblock
SB = G_SEQ * S                   # 308 tokens per superblock
MASK_NEG = -1.0e5                # additive causal mask (exp -> exact 0)


def build_program(T=T_CORE, G=G_SEQ):
    import concourse.bass as bass
    import concourse.bacc as bacc
    import concourse.mybir as mybir
    import concourse.tile as tile
    from concourse.masks import make_identity
    from contextlib import ExitStack

    f32 = mybir.dt.float32
    bf16 = mybir.dt.bfloat16
    AX = mybir.AxisListType
    OP = mybir.AluOpType
    AF = mybir.ActivationFunctionType

    SBLK = G * S
    NSB = T // SBLK
    assert NSB * SBLK == T
    NH2 = H // 2                  # 6 head pairs
    SW = NH2 * S                  # 462 score columns per bank
    # token chunks within a superblock
    chunks = []
    off = 0
    while off < SBLK:
        w = min(128, SBLK - off)
        chunks.append((off, w))
        off += w

    nc = bacc.Bacc("TRN2", target_bir_lowering=False)

    fp8 = mybir.dt.float8e4
    x_d = nc.declare_dram_parameter("x", [T, D], f32, isOutput=False)
    wq_d = nc.declare_dram_parameter("wqT", [D, D], fp8, isOutput=False)
    wk_d = nc.declare_dram_parameter("wkT", [D, D], fp8, isOutput=False)
    wv_d = nc.declare_dram_parameter("wvT", [D, D], fp8, isOutput=False)
    wo_d = nc.declare_dram_parameter("woT", [D, D], bf16, isOutput=False)
    wf1_d = nc.declare_dram_parameter("fc1T", [D, FF], fp8, isOutput=False)
    wf2_d = nc.declare_dram_parameter("fc2T", [FF, D], fp8, isOutput=False)
    qb_d = nc.declare_dram_parameter("qb", [D], f32, isOutput=False)
    kb_d = nc.declare_dram_parameter("kb", [D], f32, isOutput=False)
    vb_d = nc.declare_dram_parameter("vb", [D], f32, isOutput=False)
    ob_d = nc.declare_dram_parameter("ob", [D], f32, isOutput=False)
    f1b_d = nc.declare_dram_parameter("fc1b", [FF], f32, isOutput=False)
    f2b_d = nc.declare_dram_parameter("fc2b", [D], f32, isOutput=False)
    mask_d = nc.declare_dram_parameter("mask6", [S, SW], bf16, isOutput=False)
    out_d = nc.declare_dram_parameter("out", [T, D], f32, isOutput=True)

    with tile.TileContext(nc) as tc, ExitStack() as ctx:
        singles = ctx.enter_context(tc.tile_pool(name="singles", bufs=1))
        xpool = ctx.enter_context(tc.tile_pool(name="xpool", bufs=3))
        x2pool = ctx.enter_context(tc.tile_pool(name="x2pool", bufs=3))
        actpool = ctx.enter_context(tc.tile_pool(name="actpool", bufs=1))
        outpool = ctx.enter_context(tc.tile_pool(name="outpool", bufs=1))
        attnpool = ctx.enter_context(tc.tile_pool(name="attnpool", bufs=2))
        statpool = ctx.enter_context(tc.tile_pool(name="statpool", bufs=2))
        pspool = ctx.enter_context(tc.tile_pool(name="pspool", bufs=2, space="PSUM"))

        # ---- constants / weights ----
        wq_sb = singles.tile([128, D // 128, D], fp8)
        wk_sb = singles.tile([128, D // 128, D], fp8)
        wv_sb = singles.tile([128, D // 128, D], fp8)
        wo_sb = singles.tile([128, D // 128, D], bf16)
        wf1_sb = singles.tile([128, D // 128, FF], fp8)
        wf2_sb = singles.tile([128, FF // 128, D], fp8)
        for sb_t, dr in ((wq_sb, wq_d), (wk_sb, wk_d), (wv_sb, wv_d),
                         (wo_sb, wo_d), (wf1_sb, wf1_d), (wf2_sb, wf2_d)):
            nc.sync.dma_start(out=sb_t, in_=dr[:].rearrange("(c p) o -> p c o", p=128))

        qb_sb = singles.tile([128, D // 128], f32)
        kb_sb = singles.tile([128, D // 128], f32)
        f1b_sb = singles.tile([128, FF // 128], f32)
        f2b_sb = singles.tile([128, D // 128], f32)
        for sb_t, dr in ((qb_sb, qb_d), (kb_sb, kb_d), (f1b_sb, f1b_d),
                         (f2b_sb, f2b_d)):
            nc.sync.dma_start(out=sb_t, in_=dr[:].rearrange("(c p) -> p c", p=128))

        # free-axis biases broadcast to all 128 partitions
        ob_bc = singles.tile([128, D], f32)
        vb_bc = singles.tile([128, D], f32)
        for sb_t, dr in ((ob_bc, ob_d), (vb_bc, vb_d)):
            src = bass.AP(tensor=dr[:].tensor, offset=dr[:].offset,
                          ap=[[0, 128]] + list(dr[:].ap))
            nc.sync.dma_start(out=sb_t, in_=src)

        mask6_sb = singles.tile([S, SW], bf16)
        nc.sync.dma_start(out=mask6_sb, in_=mask_d[:])

        ident = singles.tile([128, 128], bf16)
        make_identity(nc, ident)

        ones77 = singles.tile([S, 128], bf16)
        nc.vector.memset(ones77, 1.0)

        eps_sb = singles.tile([128, 1], f32)
        nc.vector.memset(eps_sb, EPS)

        NCH = D // 128    # 6
        NFF = FF // 128   # 24

        def ln_normalize(src_tile, w, tag, bufs=2):
            """token-major [w, 768] fp32 -> normalized bf16 htok tile."""
            stats = statpool.tile([128, 3, 6], f32, tag=f"stats{tag}", name=f"stats{tag}")
            mv = statpool.tile([128, 3], f32, tag=f"mv{tag}", name=f"mv{tag}")
            xg = src_tile[:w].rearrange("p (s f) -> p s f", f=256)
            for i in range(3):
                nc.vector.bn_stats(out=stats[:w, i, :], in_=xg[:, i, :])
            nc.vector.bn_aggr(out=mv[:w, 0:2], in_=stats[:w])
            mean = mv[:w, 0:1]
            var = mv[:w, 1:2]
            lnv = mv[:w, 2:3]
            # rstd = exp(-0.5*ln(var+eps)) - stays in the natural_log_exp
            # activation-table set shared with attention's EXP.
            nc.scalar.activation(out=lnv, in_=var, func=AF.Ln,
                                 bias=eps_sb[:w], scale=1.0)
            nc.scalar.activation(out=var, in_=lnv, func=AF.Exp,
                                 bias=0.0, scale=-0.5)
            rstd = var
            htok = statpool.tile([128, D], bf16, tag=f"htok{tag}", name=f"htok{tag}",
                                 bufs=bufs)
            nc.vector.tensor_scalar(out=htok[:w], in0=src_tile[:w],
                                    scalar1=mean, scalar2=rstd,
                                    op0=OP.subtract, op1=OP.mult)
            return htok

        def seq_pieces(coff, w):
            """split token range [coff, coff+w) into per-sequence pieces of
            (src_col_in_chunk, length, dst_col) with 80-padded dst stride."""
            out_ = []
            t = coff
            while t < coff + w:
                s_ = t // S
                e = min((s_ + 1) * S, coff + w)
                out_.append((t - coff, e - t, s_ * 80 + (t - s_ * S)))
                t = e
            return out_

        def ln_transpose(htok, coff, w, dst, tag, padded=False):
            pieces = seq_pieces(coff, w) if padded else [(0, w, coff)]
            for c in range(NCH):
                ps = pspool.tile([128, 128], bf16, tag="tr", name=f"trp{tag}")
                nc.tensor.transpose(ps[:, :w], htok[:w, c * 128:(c + 1) * 128],
                                    ident[:w, :w])
                for (po_, ln_, dc_) in pieces:
                    nc.vector.tensor_copy(out=dst(c, dc_, ln_),
                                          in_=ps[:, po_:po_ + ln_])

        def stage_A(isb):
            """load x, LN1 -> hT8 feature-major fp8; then x_tok += ob (gpsimd)."""
            t0 = isb * SBLK
            hT8 = actpool.tile([128, NCH, 320], fp8, tag="hT8", name="hT8", bufs=2)
            x_tiles = []
            for (coff, w) in chunks:
                x_tok = xpool.tile([128, D], f32, tag="xtok", name="xtok")
                nc.sync.dma_start(out=x_tok[:w], in_=x_d[t0 + coff: t0 + coff + w, :])
                x_tiles.append(x_tok)
                htok = ln_normalize(x_tok, w, "A")
                ln_transpose(htok, coff, w,
                             lambda c, o, ww: hT8[:, c, o:o + ww], "A",
                             padded=True)
                # after LN consumed raw x: fold the o-proj bias into the
                # residual in place on the idle GpSimd engine.
                nc.gpsimd.tensor_tensor(out=x_tok[:w], in0=x_tok[:w],
                                        in1=ob_bc[:w], op=OP.add)
            return hT8, x_tiles

        def stage_D_chunk(ci, ctxT, x_tiles, x2_tiles):
            coff, w = chunks[ci]
            x2 = x2pool.tile([128, D], f32, tag="x2tok", name="x2tok")
            for half in range(2):
                ps = pspool.tile([128, 384], f32, tag="big", name="pso")
                for d in range(NCH):
                    nc.tensor.matmul(ps[:w], lhsT=ctxT[d][:, coff:coff + w],
                                     rhs=wo_sb[:, d, half * 384:(half + 1) * 384],
                                     start=(d == 0), stop=(d == NCH - 1))
                sl = slice(half * 384, (half + 1) * 384)
                nc.vector.tensor_tensor(out=x2[:w, sl], in0=ps[:w],
                                        in1=x_tiles[ci][:w, sl], op=OP.add)
            x2_tiles.append(x2)

        DR = mybir.MatmulPerfMode.DoubleRow
        cur = stage_A(0)
        for isb in range(NSB):
            t0 = isb * SBLK
            hT8, x_tiles = cur

            # ---- stage B: q/k projections (fp8 DoubleRow, weights 16x).
            # qT/kT inherit hT8's 80-padded per-sequence column layout. ----
            qT = [actpool.tile([128, 320], bf16, tag=f"qT{c}", name=f"qT{c}")
                  for c in range(NCH)]
            kT = [actpool.tile([128, 320], bf16, tag=f"kT{c}", name=f"kT{c}")
                  for c in range(NCH)]
            for dst, w_sb, b_sb in ((qT, wq_sb, qb_sb), (kT, wk_sb, kb_sb)):
                for c in range(NCH):
                    ps = pspool.tile([128, 320], f32, tag="big", name="psqkv")
                    for dp in range(NCH // 2):
                        nc.tensor.matmul(ps,
                                         lhsT=w_sb[:, 2 * dp:2 * dp + 2,
                                                   c * 128:(c + 1) * 128],
                                         rhs=hT8[:, 2 * dp:2 * dp + 2, :],
                                         perf_mode=DR,
                                         start=(dp == 0), stop=(dp == NCH // 2 - 1))
                    nc.vector.tensor_scalar(out=dst[c], in0=ps,
                                            scalar1=1.0 / 16,
                                            scalar2=b_sb[:, c:c + 1],
                                            op0=OP.mult, op1=OP.add)

            # ---- stage C: attention per sequence ----
            ctxT = [actpool.tile([128, SBLK], bf16, tag=f"ctxT{c}", name=f"ctxT{c}",
                                 bufs=2)
                    for c in range(NCH)]
            # fp8 feature-major LN2 activations for the DoubleRow FC1
            # (padded to 320 so the k-pair free step is 16B-aligned)
            h2T8 = actpool.tile([128, NCH, 320], fp8, tag="h2T8", name="h2T8")
            x2_tiles = []
            h2toks = []
            next_chunk = 0
            for s in range(G):
                so = s * S        # token-contiguous column base (ctxT)
                sp = s * 80       # 80-padded column base (hT8/qT/kT)
                # v for this sequence, token-major directly (swapped operands,
                # fp8 DoubleRow; the padded hT8 base keeps offsets 16B-aligned)
                vtok = attnpool.tile([S, H, HD], bf16, tag="vtok", name="vtok")
                for half in range(2):
                    psv = pspool.tile([S, 384], f32, tag="big", name="psvtok")
                    for dp in range(NCH // 2):
                        nc.tensor.matmul(psv,
                                         lhsT=hT8[:, 2 * dp:2 * dp + 2,
                                                  sp:sp + S],
                                         rhs=wv_sb[:, 2 * dp:2 * dp + 2,
                                                   half * 384:(half + 1) * 384],
                                         perf_mode=DR,
                                         start=(dp == 0), stop=(dp == NCH // 2 - 1))
                    nc.vector.scalar_tensor_tensor(
                        out=vtok[:, half * 6:(half + 1) * 6, :],
                        in0=psv, scalar=1.0 / 16,
                        in1=vb_bc[:S, half * 384:(half + 1) * 384],
                        op0=OP.mult, op1=OP.add)
                # scoresT[k, q] per head; even heads -> bank A, odd -> bank B.
                # Row-group packing: even heads live on partitions 0:64 of
                # their qT/kT chunk, odd heads on 64:128 -> pairs overlap.
                psc = [pspool.tile([S, SW], f32, tag="sc", name=f"psc{a}")
                       for a in range(2)]
                for j in range(NH2):
                    for a in range(2):
                        h = 2 * j + a
                        c, po = h // 2, 64 * (h % 2)
                        nc.tensor.matmul(psc[a][:, j * S:(j + 1) * S],
                                         lhsT=kT[c][po:po + 64, sp:sp + S],
                                         rhs=qT[c][po:po + 64, sp:sp + S],
                                         start=(j == 0), stop=False,
                                         skip_group_check=True)
                # additive causal mask via identity-matmul accumulation
                pT = attnpool.tile([S, 2, SW], bf16, tag="pT", name="pT")
                for a in range(2):
                    nc.tensor.matmul(psc[a], lhsT=ident[:S, :S], rhs=mask6_sb,
                                     start=False, stop=True, skip_group_check=True)
                    nc.scalar.activation(out=pT[:, a, :], in_=psc[a], func=AF.Exp)
                # denominators broadcast across partitions: ones.T @ pT, the
                # two banks column-packed into one PSUM tile (concurrent).
                dben = pspool.tile([128, SW], f32, tag="sc", name="dben")
                rp = attnpool.tile([128, SW], bf16, tag="rp", name="rp")
                lnd = attnpool.tile([128, SW], f32, tag="lnd", name="lnd")
                for a in range(2):
                    nc.tensor.matmul(dben[64 * a:64 * a + 64, :],
                                     lhsT=ones77[:, 64 * a:64 * a + 64],
                                     rhs=pT[:, a, :],
                                     start=True, stop=True,
                                     skip_group_check=True)
                # reciprocal as exp(-ln(x)) on the scalar engine: same table
                # set as the attention EXP, and off the busy vector engine
                # (nc.vector.reciprocal is ~6.5ns/elem - 3us per call here).
                for a in range(2):
                    sl = slice(64 * a, 64 * a + 64)
                    nc.scalar.activation(out=lnd[sl], in_=dben[sl], func=AF.Ln)
                    nc.scalar.activation(out=rp[sl], in_=lnd[sl], func=AF.Exp,
                                         scale=-1.0)
                # ctx per head pair, column-packed into [128, 77] PSUM already
                # in ctxT layout; normalization fused into the evacuation.
                for j in range(NH2):
                    ctxp = pspool.tile([128, S], f32, tag="ctxp", name="ctxp")
                    for a in range(2):
                        h = 2 * j + a
                        nc.tensor.matmul(ctxp[64 * a:64 * a + 64, :],
                                         lhsT=vtok[:, h, :],
                                         rhs=pT[:, a, j * S:(j + 1) * S],
                                         start=True, stop=True,
                                         skip_group_check=True)
                    nc.vector.tensor_tensor(out=ctxT[j][:, so:so + S], in0=ctxp,
                                            in1=rp[:, j * S:(j + 1) * S],
                                            op=OP.mult)
                # emit O-proj + residual + LN2 for chunks fully covered
                done_tokens = (s + 1) * S
                while (next_chunk < len(chunks)
                       and chunks[next_chunk][0] + chunks[next_chunk][1]
                       <= done_tokens):
                    ci = next_chunk
                    stage_D_chunk(ci, ctxT, x_tiles, x2_tiles)
                    coff, w = chunks[ci]
                    h2toks.append(ln_normalize(x2_tiles[ci], w, "E", bufs=3))
                    next_chunk += 1

            # ---- prefetch next superblock's stage A here: its scalar ln/exp
            # ops stay adjacent to attention's (same table set), its
            # transposes keep the PE warm through the E/F transition ----
            if isb + 1 < NSB:
                cur = stage_A(isb + 1)

            # E transposes (emitted after C so the in-order PE isn't blocked
            # mid-attention waiting on the LN chains)
            for ci, (coff, w) in enumerate(chunks):
                ln_transpose(h2toks[ci], coff, w,
                             lambda c, o, ww: h2T8[:, c, o:o + ww], "E")

            # ---- stage F: MLP (fp8 DoubleRow matmuls, weights 16x-scaled
            # host-side; two 128-contractions fused per instruction) ----
            DR = mybir.MatmulPerfMode.DoubleRow
            ff1_8 = actpool.tile([128, NFF, 320], fp8, tag="ff18", name="ff18")
            for f in range(NFF):
                ps = pspool.tile([128, SBLK], f32, tag="big", name="psff")
                for dp in range(NCH // 2):
                    nc.tensor.matmul(ps,
                                     lhsT=wf1_sb[:, 2 * dp:2 * dp + 2,
                                                 f * 128:(f + 1) * 128],
                                     rhs=h2T8[:, 2 * dp:2 * dp + 2, :SBLK],
                                     perf_mode=DR,
                                     start=(dp == 0), stop=(dp == NCH // 2 - 1))
                # f1 = silu(1.702*(ps/16) + 1.702*b) = 1.702*quickgelu(ps+b);
                # the 1/1.702 is folded into fc2T host-side.
                nc.scalar.activation(out=ff1_8[:, f, :SBLK], in_=ps, func=AF.Silu,
                                     bias=f1b_sb[:, f:f + 1], scale=1.702 / 16)
            # FC2 feature-major (weights stationary, tokens streaming), then
            # PE-transpose each dout chunk back to token-major with the final
            # residual fused into the PSUM evacuation add.
            o_toks = [outpool.tile([128, D], f32, tag=f"otok{ci}", name=f"otok{ci}")
                      for ci in range(len(chunks))]
            for c in range(NCH):
                ps = pspool.tile([128, SBLK], f32, tag="big", name="psf2")
                for fp in range(NFF // 2):
                    nc.tensor.matmul(ps,
                                     lhsT=wf2_sb[:, 2 * fp:2 * fp + 2,
                                                 c * 128:(c + 1) * 128],
                                     rhs=ff1_8[:, 2 * fp:2 * fp + 2, :SBLK],
                                     perf_mode=DR,
                                     start=(fp == 0), stop=(fp == NFF // 2 - 1))
                x3c = statpool.tile([128, SBLK], bf16, tag="x3", name="x3")
                nc.vector.tensor_scalar(out=x3c, in0=ps,
                                        scalar1=1.0 / 16, scalar2=f2b_sb[:, c:c + 1],
                                        op0=OP.mult, op1=OP.add)
                for ci, (coff, w) in enumerate(chunks):
                    tr = pspool.tile([128, 128], bf16, tag="tr", name="trf")
                    nc.tensor.transpose(tr[:w, :], x3c[:, coff:coff + w], ident)
                    nc.vector.tensor_tensor(
                        out=o_toks[ci][:w, c * 128:(c + 1) * 128],
                        in0=tr[:w, :],
                        in1=x2_tiles[ci][:w, c * 128:(c + 1) * 128], op=OP.add)
            for ci, (coff, w) in enumerate(chunks):
                nc.sync.dma_start(out=out_d[t0 + coff: t0 + coff + w, :],
                                  in_=o_toks[ci][:w])

    # Restrict the activation-table-set chooser to the two sets that cover
    # everything this kernel uses (ln+exp share one set; silu the other).
    # Entries keep their original indices (act_func_set_id is positional);
    # unwanted sets are just emptied so the chooser can never pick them.
    from concourse.hw_specs import get_activation_tables
    import bass_rust as _bass_rust
    _tables = list(get_activation_tables(nc.m.arch).items())
    _keep = {"natural_log_exp_and_others", "silu_and_others"}
    _tables = [(n, (f if n in _keep else set())) for (n, f) in _tables]

    def _patched_insert_act_table_loads():
        _bass_rust.insert_act_table_loads(nc, _tables)

    nc.insert_act_table_loads = _patched_insert_act_table_loads
    nc.compile()
    return nc


def prep_shared(inputs):
    """Fold LN affine params / scale constants into weights -> shared in_map entries."""
    bf = ml_dtypes.bfloat16
    f32 = np.float32
    g = {k: np.asarray(v, dtype=np.float32) for k, v in inputs.items() if k != "x"}

    # projection/MLP weights in fp8 e4m3, scaled 16x so the smallest weights
    # stay out of the subnormal floor; the 1/16 is folded into the PSUM
    # evacuations (tensor_scalar mult / SILU input scale).
    e4 = ml_dtypes.float8_e4m3
    wqT = (g["ln1_w"][:, None] * g["qw"].T * 0.125 * 16.0).astype(e4)
    wkT = (g["ln1_w"][:, None] * g["kw"].T * 16.0).astype(e4)
    wvT = (g["ln1_w"][:, None] * g["vw"].T * 16.0).astype(e4)
    woT = np.ascontiguousarray(g["ow"].T).astype(bf)
    fc1T = (g["ln2_w"][:, None] * g["fc1_w"].T * 16.0).astype(e4)
    fc2T = (g["fc2_w"].T / 1.702 * 16.0).astype(e4)

    qb = ((g["ln1_b"] @ g["qw"].T + g["qb"]) * 0.125).astype(f32)
    kb = (g["ln1_b"] @ g["kw"].T + g["kb"]).astype(f32)
    vb = (g["ln1_b"] @ g["vw"].T + g["vb"]).astype(f32)
    ob = g["ob"].astype(f32)
    fc1b = ((g["ln2_b"] @ g["fc1_w"].T + g["fc1_b"]) * 1.702).astype(f32)
    fc2b = g["fc2_b"].astype(f32)

    # additive causal mask in scoresT[k, q] orientation (k > q masked),
    # tiled 6x along q for the per-bank [77, 462] accumulation matmul.
    m1 = np.where(np.arange(S)[:, None] > np.arange(S)[None, :], MASK_NEG, 0.0)
    mask6 = np.tile(m1.astype(np.float32), (1, H // 2)).astype(bf)

    return dict(wqT=wqT, wkT=wkT, wvT=wvT, woT=woT, fc1T=fc1T, fc2T=fc2T,
                qb=qb, kb=kb, vb=vb, ob=ob, fc1b=fc1b, fc2b=fc2b, mask6=mask6)


def prep_host_inputs(inputs):
    shared = prep_shared(inputs)
    x = np.asarray(inputs["x"], dtype=np.float32)
    in_maps = []
    for c in range(N_CORES):
        xc = np.ascontiguousarray(
            x[c * BPC:(c + 1) * BPC].reshape(T_CORE, D).astype(np.float32))
        in_maps.append(dict(shared, x=xc))
    return in_maps


_CACHED_NC = None


def _get_nc():
    global _CACHED_NC
    if _CACHED_NC is None:
        _CACHED_NC = build_program()
    return _CACHED_NC


def run(inputs, trace=False):
    from concourse.bass_utils import run_bass_kernel_spmd
    nc = _get_nc()
    in_maps = prep_host_inputs(inputs)
    res = run_bass_kernel_spmd(nc, in_maps, list(range(N_CORES)), trace=trace)
    outs = [np.asarray(res.results[c]["out"], dtype=np.float32).reshape(BPC, S, D)
            for c in range(N_CORES)]
    full = np.concatenate(outs, axis=0)
    return full, res


def kernel(**inputs):
    full, _ = run(inputs, trace=False)
    return full
